# revision 1
# baseline (speedup 1.0000x reference)
"""MultiHeadGeometryAttention Trainium2 kernel.

Sharding: 8 cores = (B=2) x (N=2048 split into 4 row-chunks of 512).
Each core computes, for its 512 query rows:
  - the NxN geometry bias + side gate ONCE (shared by all 4 heads),
  - all-head attention in transposed layout S^T[j, i] so per-query terms
    fold into TensorE matmuls as augmented contractions,
  - PV with a ones-augmented V column -> softmax denominators for free,
  - the final output projection on-device (naturally un-transposed).
Host work is O(N): unit cross products, norms, layout transposes, and
concatenating the 8 row-chunk outputs.

All matmuls run in float32r (full PE rate). The geometry distance
matmuls cancel catastrophically (d2 ~ 0 from O(1) products), so their
inputs are split hi/lo: hi is 10-bit-mantissa-exact (survives f32r's
~12-bit input rounding), lo carries the residual on duplicated
contraction rows. The geometry bias is folded into the score PSUM via
an identity matmul, so the per-(head, tile) elementwise work is just
one DVE multiply and one ACT exp. Per-head tensors keep the head index
on a free dim (PE operands only allow base partitions 0/32/64).
PSUM pools are phase-scoped: the 5 geometry banks are released before
the attention phase so scores can quadruple-buffer.
"""

import math

import numpy as np

import concourse.bass as bass  # noqa: F401
import concourse.mybir as mybir
import concourse.tile as tile
from concourse import bacc
from concourse.bass import _add_dep_helper
from concourse.bass_utils import run_bass_kernel_spmd

# problem constants (fixed by the nn.Module config)
ALPHA0 = 1.0
BETA0 = 4.0
GAMMA = 0.5
SIGMA = 0.2
W_MIN, W_MAX = 0.05, 0.3
B, N, D, H = 2, 2048, 128, 4
HD = D // H  # 32
CH = 512  # query rows per core
NCORES = 8
NJT = N // 128  # 16 key tiles
NSUB = CH // 128  # 4 query subtiles
NG = 17  # geometry contraction rows (with hi/lo splits)

HALF_W = 0.5 * (W_MIN + W_MAX)  # 0.175
GATE_INV_SCALE = 1.0 / (0.25 * (W_MAX - W_MIN))  # 16.0
NEG_INV_2SIG2 = -1.0 / (2.0 * SIGMA * SIGMA)  # -12.5
SC = 1.0 / math.sqrt(HD)

F32 = mybir.dt.float32
F32R = mybir.dt.float32r
AF = mybir.ActivationFunctionType
ALU = mybir.AluOpType

_cache = {}


def _build_program(has_bk=False, has_bv=False):
    nc = bacc.Bacc(None)

    ident = nc.dram_tensor("ident", [D, D], F32, kind="ExternalInput")
    xT = nc.dram_tensor("xt", [D, N], F32, kind="ExternalInput")
    xTc = nc.dram_tensor("xtc", [D, CH], F32, kind="ExternalInput")
    geo = nc.dram_tensor("geo", [NG, N], F32, kind="ExternalInput")
    rhs = nc.dram_tensor("rhs", [NG, 7, CH], F32, kind="ExternalInput")
    wq = nc.dram_tensor("wq", [D, H, HD], F32, kind="ExternalInput")
    wk = nc.dram_tensor("wk", [HD, H, D], F32, kind="ExternalInput")
    wv = nc.dram_tensor("wv", [D, D], F32, kind="ExternalInput")
    wo = nc.dram_tensor("wo", [HD, H, D], F32, kind="ExternalInput")
    bq = nc.dram_tensor("bq", [HD, H], F32, kind="ExternalInput")
    bk = nc.dram_tensor("bk", [HD, H], F32, kind="ExternalInput")
    bv = nc.dram_tensor("bv", [HD, H], F32, kind="ExternalInput")
    ob = nc.dram_tensor("ob", [1, D], F32, kind="ExternalInput")
    ones2 = nc.dram_tensor("ones2", [2, D], F32, kind="ExternalInput")
    vones = nc.dram_tensor("vones", [128, 1], F32, kind="ExternalInput")
    out = nc.dram_tensor("out", [CH, D], F32, kind="ExternalOutput")

    with tile.TileContext(nc) as tc, nc.allow_low_precision(
        reason="float32r rounding of matmul operands is intentional"
    ):
        with (
            tc.tile_pool(name="const", bufs=1) as const,
            tc.tile_pool(name="tmp", bufs=1) as tmp,
            tc.tile_pool(name="geomp", bufs=NJT) as geomp,
            tc.tile_pool(name="gatep", bufs=NJT) as gatep,
            tc.tile_pool(name="ep", bufs=1) as ep,
            tc.tile_pool(name="stagep", bufs=6) as stagep,
            tc.tile_pool(name="bias_ps", bufs=1, space="PSUM") as bias_ps,
            tc.tile_pool(name="s_ps", bufs=3, space="PSUM") as s_ps,
            tc.tile_pool(name="pv_ps", bufs=1, space="PSUM") as pv_ps,
        ):
            # ---- load + round matmul operands to f32r (separate tiles:
            # the verifier tracks rounded-ness per memory location), chunked
            # through small staging slots so rounding pipelines with DMA ----
            _engs = [nc.vector, nc.gpsimd, nc.scalar]
            _eng_i = [0]

            def _round_copy(dst, stage):
                e = _engs[_eng_i[0] % len(_engs)]
                _eng_i[0] += 1
                if e is nc.scalar:
                    e.copy(dst, stage)
                else:
                    e.tensor_copy(dst, stage)

            def load_r(name, t, shape):
                r = const.tile(shape, F32R, tag=name, name=name + "_sb")
                stage = stagep.tile(shape, F32, tag="stage", name=name + "_st")
                nc.sync.dma_start(stage, t[...])
                _round_copy(r, stage)
                return r

            geo_sb = const.tile([NG, N], F32R, tag="geo", name="geo_sb")
            for c in range(4):
                stage = stagep.tile([NG, CH], F32, tag="stage", name="geo_st")
                nc.sync.dma_start(stage, geo[:, c * CH : (c + 1) * CH])
                _round_copy(geo_sb[:, c * CH : (c + 1) * CH], stage)
            rhs_sb = const.tile([NG, 7, CH], F32R, tag="rhs", name="rhs_sb")
            for m in range(7):
                stage = stagep.tile([NG, 1, CH], F32, tag="stage", name="rhs_st")
                nc.sync.dma_start(stage, rhs[:, m : m + 1, :])
                _round_copy(rhs_sb[:, m : m + 1, :], stage)
            ident_sb = load_r("ident", ident, [D, D])
            xTc_sb = load_r("xTc", xTc, [D, CH])
            wq_sb = load_r("wq", wq, [D, H, HD])
            wk_sb = load_r("wk", wk, [HD, H, D])
            wv_sb = load_r("wv", wv, [D, D])
            wo_sb = load_r("wo", wo, [HD, H, D])
            ob_sb = load_r("ob", ob, [1, D])
            xT_sb = const.tile([D, N], F32R, tag="xT", name="xT_sb")
            for c in range(4):
                stage = stagep.tile([D, CH], F32, tag="stage", name="xT_st")
                nc.sync.dma_start(stage, xT[:, c * CH : (c + 1) * CH])
                _round_copy(xT_sb[:, c * CH : (c + 1) * CH], stage)

            bq_sb = const.tile([HD, H], F32, tag="bq", name="bq_sb")
            nc.sync.dma_start(bq_sb, bq[:, :])
            if has_bk:
                bk_sb = const.tile([HD, H], F32, tag="bk", name="bk_sb")
                nc.sync.dma_start(bk_sb, bk[:, :])
            if has_bv:
                bv_sb = const.tile([HD, H], F32, tag="bv", name="bv_sb")
                nc.sync.dma_start(bv_sb, bv[:, :])
            ones_sb = load_r("ones2", ones2, [2, D])

            qT = const.tile([HD, H, CH], F32R, tag="qT", name="qT")
            V = const.tile([128, NJT, H, HD + 1], F32R, tag="V", name="V")
            A_bc = const.tile([128, CH], F32, tag="A_bc", name="A_bc")
            E_bc = const.tile([128, CH], F32, tag="E_bc", name="E_bc")
            headcat = const.tile([HD, H, CH], F32R, tag="headcat", name="headcat")

            # ---- projections ----
            for h in range(H):
                qps = bias_ps.tile([HD, CH], F32, tag="b0", name="qps")
                nc.tensor.matmul(
                    qps, lhsT=wq_sb[:, h, :], rhs=xTc_sb, start=True, stop=True
                )
                nc.scalar.activation(
                    qT[:, h, :], qps, AF.Identity, bias=bq_sb[:, h : h + 1], scale=SC
                )
            vstage = stagep.tile([128, 1], F32, tag="stage", name="vones_st")
            nc.sync.dma_start(vstage, vones[:, :])
            nc.vector.tensor_copy(
                V[:, :, :, HD : HD + 1],
                vstage[:, None, None, :].to_broadcast([128, NJT, H, 1]),
            )
            for jt in range(NJT):
                vps = bias_ps.tile([128, D], F32, tag="b1", name="vps")
                nc.tensor.matmul(
                    vps,
                    lhsT=xT_sb[:, jt * 128 : (jt + 1) * 128],
                    rhs=wv_sb,
                    start=True,
                    stop=True,
                )
                nc.vector.tensor_copy(
                    out=V[:, jt, :, 0:HD],
                    in_=vps.rearrange("p (h d) -> p h d", h=H),
                )
            abc_ps = bias_ps.tile([128, CH], F32, tag="b2", name="abc_ps")
            nc.tensor.matmul(
                abc_ps,
                lhsT=ones_sb[:, 0:128],
                rhs=rhs_sb[0:2, 5, :],
                start=True,
                stop=True,
            )
            nc.scalar.copy(A_bc, abc_ps)
            ebc_ps = bias_ps.tile([128, CH], F32, tag="b3", name="ebc_ps")
            nc.tensor.matmul(
                ebc_ps,
                lhsT=ones_sb[0:1, 0:128],
                rhs=rhs_sb[0:1, 6, :],
                start=True,
                stop=True,
            )
            nc.scalar.copy(E_bc, ebc_ps)

            # qk_h = Wk_h^T @ qT_h for every head, up front
            qk4 = const.tile([D, H, CH], F32R, tag="qk4", name="qk4")
            bkq4 = None
            for h in range(H):
                qkps = bias_ps.tile([D, CH], F32, tag="b2", name="qkps")
                nc.tensor.matmul(
                    qkps, lhsT=wk_sb[:, h, :], rhs=qT[:, h, :], start=True, stop=True
                )
                nc.scalar.copy(qk4[:, h, :], qkps)
            if has_bk:
                bkq4 = const.tile([1, H, CH], F32R, tag="bkq4", name="bkq4")
                for h in range(H):
                    bkq_ps = bias_ps.tile([1, CH], F32, tag="b3", name="bkq_ps")
                    nc.tensor.matmul(
                        bkq_ps,
                        lhsT=bk_sb[:, h : h + 1].bitcast(F32R),
                        rhs=qT[:, h, :],
                        start=True,
                        stop=True,
                    )
                    nc.vector.tensor_copy(bkq4[:, h, :], bkq_ps)

            # ---- geometry bias + gate (shared across heads), in halves so
            # sigmoids batch (Exp/Sigmoid are in different ACT table sets)
            # while phase B can start on the first half's tiles ----
            geom_tiles = []
            gate_tiles = []
            HALF = NJT // 2
            def emit_bias_half(half):
                caff_last = None
                caff_last = None
                for jt in range(half * HALF, (half + 1) * HALF):
                    lhs = geo_sb[:, jt * 128 : (jt + 1) * 128]
                    dp_ps = bias_ps.tile([128, CH], F32, tag="b0", name="dp_ps")
                    nc.tensor.matmul(
                        dp_ps, lhsT=lhs, rhs=rhs_sb[:, 0, :], start=True, stop=True
                    )
                    d2_ps = bias_ps.tile([128, CH], F32, tag="b1", name="d2_ps")
                    nc.tensor.matmul(
                        d2_ps, lhsT=lhs, rhs=rhs_sb[:, 1, :], start=True, stop=True
                    )
                    ns_ps = bias_ps.tile([128, CH], F32, tag="b2", name="ns_ps")
                    nc.tensor.matmul(
                        ns_ps, lhsT=lhs, rhs=rhs_sb[:, 2, :], start=True, stop=True
                    )
                    cd_ps = bias_ps.tile([128, CH], F32, tag="b3", name="cd_ps")
                    nc.tensor.matmul(
                        cd_ps, lhsT=lhs, rhs=rhs_sb[:, 4, :], start=True, stop=True
                    )

                    dp2 = tmp.tile([128, CH], F32, tag="dp2", name="dp2", bufs=2)
                    nc.scalar.activation(dp2, dp_ps, AF.Square)
                    cda = tmp.tile([128, CH], F32, tag="cda", name="cda")
                    nc.scalar.activation(cda, cd_ps, AF.Abs)

                    lat_ps = bias_ps.tile([128, CH], F32, tag="b3", name="lat_ps")
                    nc.tensor.matmul(
                        lat_ps, lhsT=lhs, rhs=rhs_sb[:, 3, :], start=True, stop=True
                    )

                    pd2 = tmp.tile([128, CH], F32, tag="pd2", name="pd2", bufs=2)
                    nc.vector.tensor_tensor(pd2, d2_ps, dp2, ALU.subtract)
                    m1 = tmp.tile([128, CH], F32, tag="m1", name="m1")
                    nc.vector.tensor_mul(m1, dp2, A_bc)
                    # m2 = min(-50*pd2, 0) = -50*relu(pd2); geomA = m1 + m2
                    m2 = tmp.tile([128, CH], F32, tag="m2", name="m2")
                    nc.gpsimd.tensor_scalar(
                        m2, pd2, BETA0 * NEG_INV_2SIG2, 0.0, ALU.mult, ALU.min
                    )
                    geomA = tmp.tile([128, CH], F32, tag="geomA", name="geomA")
                    nc.gpsimd.tensor_add(geomA, m1, m2)

                    caff = tmp.tile([128, CH], F32, tag="caff", name="caff")
                    caff_last = nc.scalar.activation(caff, cda, AF.Exp, scale=-1.0)
                    g1 = tmp.tile([128, CH], F32, tag="g1", name="g1")
                    nc.vector.tensor_mul(g1, caff, ns_ps)
                    geom_t = geomp.tile([128, CH], F32R, tag="geom", name="geom_t")
                    nc.vector.tensor_add(geom_t, geomA, g1)

                    lata = tmp.tile([128, CH], F32, tag="lata", name="lata")
                    nc.scalar.activation(lata, lat_ps, AF.Abs)
                    gate_t = gatep.tile([128, CH], F32, tag="gate", name="gate_t")
                    nc.gpsimd.tensor_sub(gate_t, E_bc, lata)

                    geom_tiles.append(geom_t)
                    gate_tiles.append(gate_t)

                for jt in range(half * HALF, (half + 1) * HALF):
                    si = nc.scalar.activation(
                        gate_tiles[jt],
                        gate_tiles[jt],
                        AF.Sigmoid,
                        scale=GATE_INV_SCALE,
                    )
                    _add_dep_helper(
                        si.ins, caff_last.ins, sync=False, reason="batch sigmoids"
                    )

            # ---- attention, interleaved with the second bias half ----
            pvts = {}

            def attn_steps(h, jts):
                for jt in jts:
                    sps_t = s_ps.tile([128, CH], F32, tag="s", name="sps_t")
                    nc.tensor.matmul(
                        sps_t,
                        lhsT=xT_sb[:, jt * 128 : (jt + 1) * 128],
                        rhs=qk4[:, h, :],
                        start=True,
                        stop=False,
                    )
                    if has_bk:
                        nc.tensor.matmul(
                            sps_t,
                            lhsT=ones_sb[0:1, 0:128],
                            rhs=bkq4[:, h, :],
                            start=False,
                            stop=False,
                        )
                    # accumulate geom into the scores on PE
                    nc.tensor.matmul(
                        sps_t,
                        lhsT=ident_sb,
                        rhs=geom_tiles[jt],
                        start=False,
                        stop=True,
                    )
                    s2 = ep.tile([128, CH], F32, tag="s2", name="s2", bufs=6)
                    nc.vector.tensor_mul(s2, sps_t, gate_tiles[jt])
                    e_t = ep.tile([128, CH], F32R, tag="e", name="e_t", bufs=6)
                    nc.scalar.activation(e_t, s2, AF.Exp)
                    nc.tensor.matmul(
                        pvts[h],
                        lhsT=V[:, jt, h, :],
                        rhs=e_t,
                        start=(jt == 0),
                        stop=(jt == NJT - 1),
                    )

            def attn_finish(h):
                pvt = pvts[h]
                recip = tmp.tile([1, CH], F32R, tag="recip", name="recip")
                nc.vector.reciprocal(recip, pvt[HD : HD + 1, :])
                bc_ps = s_ps.tile([HD, CH], F32, tag="s", name="bc_ps")
                nc.tensor.matmul(
                    bc_ps, lhsT=ones_sb[0:1, 0:HD], rhs=recip, start=True, stop=True
                )
                bc_sb = tmp.tile([HD, CH], F32, tag="bc", name="bc_sb")
                nc.scalar.copy(bc_sb, bc_ps)
                nc.vector.tensor_mul(headcat[:, h, :], pvt[0:HD, :], bc_sb)
                if has_bv:
                    # + v bias (A is row-stochastic after normalize)
                    nc.scalar.activation(
                        headcat[:, h, :],
                        headcat[:, h, :],
                        AF.Identity,
                        bias=bv_sb[:, h : h + 1],
                    )

            emit_bias_half(0)
            emit_bias_half(1)
            for h in range(H):
                pvts[h] = pv_ps.tile([HD + 1, CH], F32, tag="pv", name="pvt")
                attn_steps(h, range(NJT))
                attn_finish(h)

            # ---- final projection: out[i, :] = sum_h headcat_h^T @ wo_h + ob ----
            for s in range(NSUB):
                fps = s_ps.tile([128, D], F32, tag="s", name="fps")
                for h in range(H):
                    nc.tensor.matmul(
                        fps,
                        lhsT=headcat[:, h, s * 128 : (s + 1) * 128],
                        rhs=wo_sb[:, h, :],
                        start=(h == 0),
                        stop=False,
                    )
                nc.tensor.matmul(
                    fps, lhsT=ones_sb[0:1, 0:128], rhs=ob_sb, start=False, stop=True
                )
                f_sb = tmp.tile([128, D], F32, tag="f", name="f_sb")
                nc.vector.tensor_copy(f_sb, fps)
                nc.sync.dma_start(out[s * 128 : (s + 1) * 128, :], f_sb)

    nc.finalize()
    return nc


def _split_hi_lo(v, bits=10):
    """Split fp32 array into a `bits`-mantissa-exact hi part and residual."""
    v = np.asarray(v, np.float32)
    m, e = np.frexp(v.astype(np.float64))
    hi = (np.round(m * (1 << bits)) / (1 << bits) * np.exp2(e)).astype(np.float32)
    lo = (v.astype(np.float64) - hi).astype(np.float32)
    return hi, lo


def _prep_core_inputs(inputs, core):
    b, ch = core // 4, core % 4
    i0 = ch * CH
    x = np.ascontiguousarray(inputs["x"][b], np.float32)  # [N, D]
    pdir = np.ascontiguousarray(inputs["principal_dir"][b], np.float32)
    nrm = np.ascontiguousarray(inputs["normals"][b], np.float32)
    crv = inputs["curvature"][b].astype(np.float32)
    dens = inputs["density"][b].astype(np.float32)
    lin = inputs["linearity"][b].astype(np.float32)
    qkv_w = inputs["qkv_w"].astype(np.float32)
    qkv_b = inputs["qkv_b"].astype(np.float32)
    out_w = inputs["out_w"].astype(np.float32)

    xyz = x[:, :3]
    n2 = (xyz.astype(np.float64) ** 2).sum(-1).astype(np.float32)
    cr = np.cross(pdir, nrm)
    side = cr / (np.linalg.norm(cr, axis=-1, keepdims=True) + 1e-8)
    rowdot = (xyz * pdir).sum(-1)
    rowsidedot = (xyz * side).sum(-1)

    xhi, xlo = _split_hi_lo(xyz)
    n2hi, n2lo = _split_hi_lo(n2)
    phi, plo = _split_hi_lo(pdir)
    shi, slo = _split_hi_lo(side)
    rdhi, rdlo = _split_hi_lo(rowdot)
    rshi, rslo = _split_hi_lo(rowsidedot)
    alpha = NEG_INV_2SIG2 * ALPHA0 * (1.0 - lin)
    ahi, alo = _split_hi_lo(alpha)

    # GEO rows: 0-2 xhi_j, 3-5 xlo_j, 6 n2hi_j, 7 n2lo_j, 8 ones,
    #           9-11 gamma*dens*normals_j, 12 crv_j, 13-15 xhi_j dup, 16 ones dup
    geo = np.zeros((NG, N), np.float32)
    geo[0:3] = xhi.T
    geo[3:6] = xlo.T
    geo[6] = n2hi
    geo[7] = n2lo
    geo[8] = 1.0
    geo[9:12] = (GAMMA * dens)[None, :] * nrm.T
    geo[12] = crv
    geo[13:16] = xhi.T
    geo[16] = 1.0

    rhs = np.zeros((NG, 7, N), np.float32)
    # m0: d_par = rowdot_i - x_j . pdir_i
    rhs[0:3, 0] = -phi.T
    rhs[3:6, 0] = -phi.T
    rhs[13:16, 0] = -plo.T
    rhs[8, 0] = rdhi
    rhs[16, 0] = rdlo
    # m1: d2 = n2_i + n2_j - 2 x_j . x_i
    rhs[0:3, 1] = -2.0 * xhi.T
    rhs[3:6, 1] = -2.0 * xhi.T
    rhs[13:16, 1] = -2.0 * xlo.T
    rhs[6, 1] = 1.0
    rhs[7, 1] = 1.0
    rhs[8, 1] = n2hi
    rhs[16, 1] = n2lo
    # m2: nsim' = (gamma*dens_j*normals_j) . normals_i
    rhs[9:12, 2] = nrm.T
    # m3: lateral = rowsidedot_i - x_j . side_i
    rhs[0:3, 3] = -shi.T
    rhs[3:6, 3] = -shi.T
    rhs[13:16, 3] = -slo.T
    rhs[8, 3] = rshi
    rhs[16, 3] = rslo
    # m4: curv_i - curv_j
    rhs[8, 4] = crv
    rhs[12, 4] = -1.0
    # m5: alpha' broadcast rows (hi, lo); m6: eff_half broadcast
    rhs[0, 5] = ahi
    rhs[1, 5] = alo
    rhs[0, 6] = HALF_W * (0.5 + dens)

    xT = np.ascontiguousarray(x.T)
    # per-head weight layouts: wq [D, H, HD], wk/wo [HD, H, D], biases [HD, H]
    wq_a = np.ascontiguousarray(qkv_w[:, 0:D].reshape(D, H, HD))
    wk_a = np.ascontiguousarray(
        qkv_w[:, D : 2 * D].reshape(D, H, HD).transpose(2, 1, 0)
    )
    wo_a = np.ascontiguousarray(out_w.reshape(H, HD, D).transpose(1, 0, 2))
    bq_a = np.ascontiguousarray((qkv_b[0:D] * SC).reshape(H, HD).T)
    bk_a = np.ascontiguousarray(qkv_b[D : 2 * D].reshape(H, HD).T)
    bv_a = np.ascontiguousarray(qkv_b[2 * D : 3 * D].reshape(H, HD).T)
    return {
        "ident": np.eye(D, dtype=np.float32),
        "xt": xT,
        "xtc": np.ascontiguousarray(xT[:, i0 : i0 + CH]),
        "geo": geo,
        "rhs": np.ascontiguousarray(rhs[:, :, i0 : i0 + CH]),
        "wq": wq_a,
        "wk": wk_a,
        "wv": np.ascontiguousarray(qkv_w[:, 2 * D : 3 * D]),
        "wo": wo_a,
        "bq": bq_a,
        "bk": bk_a,
        "bv": bv_a,
        "ob": np.ascontiguousarray(inputs["out_b"].astype(np.float32)[None, :]),
        "ones2": np.ones((2, D), np.float32),
        "vones": np.ones((128, 1), np.float32),
    }


def _run(inputs, trace=False):
    has_bk = bool(np.any(inputs["qkv_b"][D : 2 * D]))
    has_bv = bool(np.any(inputs["qkv_b"][2 * D : 3 * D]))
    key = ("nc", has_bk, has_bv)
    if key not in _cache:
        _cache[key] = _build_program(has_bk, has_bv)
    nc = _cache[key]
    in_maps = [_prep_core_inputs(inputs, c) for c in range(NCORES)]
    res = run_bass_kernel_spmd(nc, in_maps, core_ids=list(range(NCORES)), trace=trace)
    full = np.empty((B, N, D), np.float32)
    for c in range(NCORES):
        b, ch = c // 4, c % 4
        full[b, ch * CH : (ch + 1) * CH, :] = res.results[c]["out"]
    return full, res


def kernel(**inputs):
    out, _ = _run(inputs)
    return out



# revision 38
# speedup vs baseline: 1.5989x; 1.5989x over previous
"""MultiHeadGeometryAttention Trainium2 kernel (v3).

Sharding: 8 cores = (B=2) x (N=2048 split into 4 query chunks of 512).
Each core computes the NxN geometry bias + side gate once for its 512
queries (shared by all 4 heads), then all-head attention in transposed
layout S^T[j, i] so the PV matmul contracts over keys on partitions.

Key structure (156 us baseline -> this kernel):
  - exp(-|crv_i - crv_j|) * normal_sim folded into the geometry matmul
    as a rank-64 separable expansion (PE contraction rows are free).
  - aniso term = (A2'/50) dp^2 - d2 with sqrt(A2'/50) folded into the
    d_par rhs; bias accumulates in ONE PSUM bank (ACT Square writes
    dp'^2, the -d2 / expansion matmuls accumulate on top); one eviction
    per tile, folded into scores via a 50*I fp16 identity matmul.
  - Gate sigmoid computed as (1 + tanh(-8*(|lat|-E)))/2 so every ACT
    function (Exp/Square/Abs/Tanh/Copy) lives in ONE table set -> no
    table reloads -> geometry and attention fully interleave per key
    tile inside 8 PSUM banks: geometry pa+pb (pa reused for lat),
    score half-groups 2x2, PV accumulators 2 (4 heads at partition
    bases 0/64).
  - Scores: one batched scalar_tensor_tensor per 2-head half (gate
    broadcast over heads), one batched exp per (jt, 4 heads) with bias
    -2 so e^s stays in fp16 range (cancels in softmax), PV pipelined
    one jt behind so PE never waits on the exp.
  - All tensors ship fp16 (hi/lo split hi parts fp16-exact) in few
    packed DMA blobs; fp16 matmuls run at full PE rate.
"""

import math

import numpy as np

import concourse.bass as bass  # noqa: F401
import concourse.mybir as mybir
import concourse.tile as tile
from concourse import bacc
from concourse.bass_utils import run_bass_kernel_spmd

# problem constants (fixed by the nn.Module config)
ALPHA0 = 1.0
BETA0 = 4.0
GAMMA = 0.5
SIGMA = 0.2
W_MIN, W_MAX = 0.05, 0.3
B, N, D, H = 2, 2048, 128, 4
HD = D // H  # 32
CH = 512  # query rows per core
NCORES = 8
NJT = N // 128  # 16 key tiles
NSUB = CH // 128  # 4 query subtiles

HALF_W = 0.5 * (W_MIN + W_MAX)  # 0.175
GATE_INV_SCALE = 1.0 / (0.25 * (W_MAX - W_MIN))  # 16.0
SC = 1.0 / math.sqrt(HD)
FOLD = 50.0  # geom tile carries bias/FOLD; folded back via FOLD*I matmul

R_EXP = 37  # rank of the exp(-|ci-cj|) separable expansion (geo rows 17..127)

LAG = 5  # attention trails geometry by this many key tiles
# geometry eviction engine per jt: True -> ACT, False -> DVE
GEOM_ACT = [False] * 16

# blob16 column offsets (keys are rotated per-core so xtc = xt[:, 0:CH])
C_I50, C_WV, C_WK, C_WO, C_ONES, C_E4 = 0, 128, 256, 768, 1280, 1408
B16C = 1536
# blob32: rhsm0 0-512, bq 512-516, bk 516-520, bv 520-524, ob row0 524-652
B32C = 652

F32 = mybir.dt.float32
F32R = mybir.dt.float32r
F16 = mybir.dt.float16
AF = mybir.ActivationFunctionType
ALU = mybir.AluOpType

_cache = {}


def _build_program(has_bk=False, has_bv=False):
    nc = bacc.Bacc(None)

    blob16 = nc.dram_tensor("blob16", [128, B16C], F16, kind="ExternalInput")
    blob32 = nc.dram_tensor("blob32", [128, B32C], F32, kind="ExternalInput")
    xt = nc.dram_tensor("xt", [D, N], F16, kind="ExternalInput")
    geo = nc.dram_tensor("geo", [128, N], F16, kind="ExternalInput")
    rhs = nc.dram_tensor("rhs", [128, 4, CH], F16, kind="ExternalInput")
    out = nc.dram_tensor("out", [128, NSUB, D], F32, kind="ExternalOutput")

    with tile.TileContext(nc) as tc, nc.allow_low_precision(
        reason="fp16 operands and f32r rounding are intentional"
    ):
        with (
            tc.tile_pool(name="const", bufs=1) as const,
            tc.tile_pool(name="tmp", bufs=1) as tmp,
            tc.tile_pool(name="s2p", bufs=3) as s2p,
            tc.tile_pool(name="ep", bufs=3) as ep,
            tc.tile_pool(name="gprep", bufs=2) as gprep,
            tc.tile_pool(name="latp", bufs=2) as latp,
        ):
            geo_sb = const.tile([128, N], F16, tag="geo", name="geo_sb")
            nc.sync.dma_start(geo_sb, geo[...])
            b32 = const.tile([128, B32C], F32, tag="b32", name="b32")
            nc.sync.dma_start(b32, blob32[...])
            rhs_sb = const.tile([128, 4, CH], F16, tag="rhs", name="rhs_sb")
            nc.sync.dma_start(rhs_sb, rhs[...])
            b16 = const.tile([128, B16C], F16, tag="b16", name="b16")
            nc.sync.dma_start(b16, blob16[...])
            xt_sb = const.tile([D, N], F16, tag="xt", name="xt_sb")
            nc.sync.dma_start(xt_sb, xt[...])

            xtc_sb = xt_sb[:, 0:CH]
            i50_sb = b16[:, C_I50 : C_I50 + D]
            wv_sb = b16[:, C_WV : C_WV + D]
            ones_sb = b16[0:1, C_ONES : C_ONES + D]

            rhsm0_sb = const.tile([128, CH], F32R, tag="rhsm0r", name="rhsm0_sb")
            nc.vector.tensor_copy(rhsm0_sb, b32[:, 0:CH])
            geo_r = const.tile([128, N], F32R, tag="geor", name="geo_r")
            nc.vector.tensor_copy(geo_r, geo_sb)
            ob_sb = const.tile([1, D], F16, tag="ob", name="ob_sb")
            nc.gpsimd.tensor_copy(ob_sb, b32[0:1, 524 : 524 + D])

            nb2 = const.tile([128, 1], F32, tag="nb2", name="nb2")
            nc.gpsimd.memset(nb2, -2.0)

            qk4 = const.tile([D, H, CH], F16, tag="qk4", name="qk4")
            V = const.tile([128, NJT, H, HD + 1], F16, tag="V", name="V")
            E16 = const.tile([128, CH], F16, tag="E16", name="E16")
            geom16 = const.tile([128, NJT, CH], F16, tag="geom16", name="geom16")
            gate16 = const.tile([128, NJT, CH], F16, tag="gate16", name="gate16")
            headcat = const.tile([HD, H, CH], F16, tag="headcat", name="headcat")

            nc.gpsimd.memset(V[:, :, :, HD : HD + 1], 1.0)

            # ---- main loop: geometry and attention interleaved ----
            with (
                tc.tile_pool(name="a_pv", bufs=1, space="PSUM") as a_pv,
            ):
                pv0 = a_pv.tile([128, CH], F32, tag="pv0", name="pv0")
                pv1 = a_pv.tile([128, CH], F32, tag="pv1", name="pv1")
                pvb = [pv0, pv1]
                prev = [None]

                with (
                    tc.tile_pool(name="g_pa", bufs=1, space="PSUM") as g_pa,
                    tc.tile_pool(name="g_pb", bufs=1, space="PSUM") as g_pb,
                    tc.tile_pool(name="a_sg", bufs=2, space="PSUM") as a_sg,
                ):
                    gpre = [None]

                    # qk4_h = M_h^T @ x_chunk with M_h = SC * Wq_h Wk_h^T
                    # host-folded (kills the q->qT->k chain on device)
                    for h in range(2):
                        sgk = a_sg.tile([128, 2, CH], F32, tag="sg",
                                        name="sgk")
                        for hh in range(2):
                            nc.tensor.matmul(
                                sgk[:, hh, :],
                                lhsT=b16[:, C_WK + (2 * h + hh) * D
                                         : C_WK + (2 * h + hh + 1) * D],
                                rhs=xtc_sb,
                                start=True,
                                stop=True,
                            )
                        nc.vector.tensor_copy(
                            qk4[:, 2 * h : 2 * h + 2, :], sgk
                        )
                    sge = a_sg.tile([128, 2, CH], F32, tag="sg", name="sge")
                    nc.tensor.matmul(
                        sge[:, 0, :], lhsT=ones_sb, rhs=rhs_sb[0:1, 3, :],
                        start=True, stop=True,
                    )
                    nc.scalar.copy(E16, sge[:, 0, :])

                    def emit_g(jt):
                        j0 = jt * 128
                        lhs = geo_sb[:, j0 : j0 + 128]
                        pa1 = g_pa.tile([128, CH], F32, tag="pa", name="pa1")
                        nc.tensor.matmul(pa1, lhsT=geo_r[:, j0 : j0 + 128],
                                         rhs=rhsm0_sb, start=True, stop=True)
                        pb = g_pb.tile([128, CH], F32, tag="pb", name="pb")
                        nc.scalar.activation(pb, pa1, AF.Square)
                        pa2 = g_pa.tile([128, CH], F32, tag="pa", name="pa2")
                        nc.tensor.matmul(pa2, lhsT=lhs, rhs=rhs_sb[:, 2, :],
                                         start=True, stop=True)
                        nc.tensor.matmul(
                            pb, lhsT=lhs, rhs=rhs_sb[:, 0, :], start=False,
                            stop=False, skip_group_check=True,
                        )
                        nc.tensor.matmul(
                            pb, lhsT=lhs, rhs=rhs_sb[:, 1, :], start=False,
                            stop=True, skip_group_check=True,
                        )
                        if GEOM_ACT[jt]:
                            nc.scalar.copy(geom16[:, jt, :], pb)
                        else:
                            nc.vector.tensor_copy(geom16[:, jt, :], pb)
                        lata = latp.tile([128, CH], F16, tag="lata",
                                         name="lata")
                        nc.scalar.activation(lata, pa2, AF.Abs)
                        if jt % 4 == 0:
                            gpre[0] = gprep.tile([128, 4, CH], F16, tag="gp",
                                                 name="gpre")
                        nc.vector.scalar_tensor_tensor(
                            gpre[0][:, jt % 4, :], lata, 0.0, E16, ALU.bypass,
                            ALU.subtract,
                        )
                        if jt % 4 == 3:
                            g = jt // 4
                            tgrp = latp.tile([128, 4, CH], F16, tag="tgrp",
                                             name="tgrp")
                            nc.scalar.activation(
                                tgrp, gpre[0], AF.Tanh,
                                scale=-0.5 * GATE_INV_SCALE,
                            )
                            # gate = 0.5 * (tanh + 1)
                            nc.vector.tensor_scalar(
                                gate16[:, g * 4 : g * 4 + 4, :], tgrp, 1.0,
                                0.5, ALU.add, ALU.mult,
                            )

                    def emit_a(jt):
                        s2 = s2p.tile([128, H, CH], F16, tag="s2", name="s2")
                        for half in range(2):
                            sg = a_sg.tile([128, 2, CH], F32, tag="sg",
                                           name="sg")
                            for hh in range(2):
                                h = 2 * half + hh
                                nc.tensor.matmul(
                                    sg[:, hh, :],
                                    lhsT=xt_sb[:, jt * 128 : (jt + 1) * 128],
                                    rhs=qk4[:, h, :],
                                    start=True,
                                    stop=False,
                                )
                                nc.tensor.matmul(
                                    sg[:, hh, :],
                                    lhsT=i50_sb,
                                    rhs=geom16[:, jt, :],
                                    start=False,
                                    stop=True,
                                    skip_group_check=True,
                                )
                            nc.vector.scalar_tensor_tensor(
                                s2[:, 2 * half : 2 * half + 2, :],
                                sg,
                                0.0,
                                gate16[:, jt : jt + 1, :].to_broadcast(
                                    [128, 2, CH]
                                ),
                                ALU.bypass,
                                ALU.mult,
                            )
                        e = ep.tile([128, H, CH], F16, tag="e", name="e")
                        nc.scalar.activation(e, s2, AF.Exp, bias=nb2)
                        if prev[0] is not None:
                            emit_pv(*prev[0])
                        prev[0] = (jt, e)

                    def emit_pv(jt, e):
                        for h in range(H):
                            b0 = (h % 2) * 64
                            nc.tensor.matmul(
                                pvb[h // 2][b0 : b0 + HD + 1, :],
                                lhsT=V[:, jt, h, :],
                                rhs=e[:, h, :],
                                start=(jt == 0),
                                stop=(jt == NJT - 1),
                                skip_group_check=True,
                            )

                    def emit_v(vj):
                        sg = a_sg.tile([128, 2, CH], F32, tag="sg", name="sgv")
                        nc.tensor.matmul(
                            sg[:, 0, 0:D],
                            lhsT=xt_sb[:, vj * 128 : (vj + 1) * 128],
                            rhs=wv_sb,
                            start=True,
                            stop=True,
                        )
                        nc.vector.tensor_copy(
                            out=V[:, vj, :, 0:HD],
                            in_=sg[:, 0, 0:D].rearrange("p (h d) -> p h d",
                                                        h=H),
                        )

                    vper = (NJT + LAG - 1) // LAG  # V-projections per G step
                    for jt in range(NJT):
                        emit_g(jt)
                        if jt < LAG:
                            for vj in range(jt * vper,
                                            min((jt + 1) * vper, NJT)):
                                emit_v(vj)
                        else:
                            emit_a(jt - LAG)
                    for jt in range(NJT - LAG, NJT):
                        emit_a(jt)
                    emit_pv(*prev[0])

                # ---- finish: normalize, concat heads, project ----
                with tc.tile_pool(name="f_ps", bufs=2, space="PSUM") as f_ps:
                    recips = []
                    for h in range(H):
                        b0 = (h % 2) * 64
                        recip = tmp.tile([1, CH], F16, tag="recip",
                                         name="recip", bufs=4)
                        nc.vector.reciprocal(
                            recip, pvb[h // 2][b0 + HD : b0 + HD + 1, :]
                        )
                        recips.append(recip)
                    bc16s = []
                    for pair in range(2):
                        bc_ps = f_ps.tile([2 * HD, CH], F32, tag="bc",
                                          name="bc_ps")
                        for hh in range(2):
                            nc.tensor.matmul(
                                bc_ps[hh * HD : (hh + 1) * HD, :],
                                lhsT=ones_sb[0:1, 0:HD],
                                rhs=recips[2 * pair + hh],
                                start=True,
                                stop=True,
                            )
                        bc16 = tmp.tile([2 * HD, CH], F16, tag="bc16",
                                        name="bc16", bufs=2)
                        nc.scalar.copy(bc16, bc_ps)
                        bc16s.append(bc16)
                    for h in range(H):
                        b0 = (h % 2) * 64
                        nc.vector.scalar_tensor_tensor(
                            headcat[:, h, :],
                            pvb[h // 2][b0 : b0 + HD, :],
                            0.0,
                            bc16s[h // 2][(h % 2) * HD : (h % 2 + 1) * HD, :],
                            ALU.bypass,
                            ALU.mult,
                        )
                        if has_bv:
                            nc.scalar.activation(
                                headcat[:, h, :], headcat[:, h, :],
                                AF.Identity,
                                bias=b32[0:HD, 520 + h : 521 + h],
                            )

                    f_all = tmp.tile([128, NSUB, D], F32, tag="fall",
                                     name="f_all")
                    for s in range(NSUB):
                        fps = f_ps.tile([128, D], F32, tag="f", name="fps")
                        for h in range(H):
                            nc.tensor.matmul(
                                fps,
                                lhsT=headcat[:, h, s * 128 : (s + 1) * 128],
                                rhs=b16[0:HD, C_WO + h * D : C_WO + (h + 1) * D],
                                start=(h == 0),
                                stop=False,
                            )
                        nc.tensor.matmul(
                            fps, lhsT=ones_sb, rhs=ob_sb, start=False,
                            stop=True,
                        )
                        if s % 2:
                            nc.scalar.copy(f_all[:, s, :], fps)
                        else:
                            nc.vector.tensor_copy(f_all[:, s, :], fps)
                    nc.sync.dma_start(out[...], f_all)

    nc.finalize()
    return nc


def _split_hi_lo(v):
    """Split fp32 array into an fp16-exact hi part and the fp32 residual."""
    v = np.asarray(v, np.float32)
    hi = v.astype(np.float16).astype(np.float32)
    lo = (v.astype(np.float64) - hi).astype(np.float32)
    return hi, lo


_exp_basis = None


def _get_exp_basis():
    """Separable rank-R_EXP expansion of exp(-|a-b|) on [0,1]^2."""
    global _exp_basis
    if _exp_basis is None:
        g = np.linspace(0.0, 1.0, 2048)
        K = np.exp(-np.abs(g[:, None] - g[None, :]))
        U, s, Vt = np.linalg.svd(K)
        r = R_EXP
        FI = U[:, :r] * np.sqrt(s[:r])
        GJ = Vt[:r].T * np.sqrt(s[:r])
        _exp_basis = (g, FI, GJ)
    return _exp_basis


def _prep_core_inputs(inputs, core):
    b, ch = core // 4, core % 4
    i0 = ch * CH
    x = np.ascontiguousarray(inputs["x"][b], np.float32)  # [N, D]
    pdir = np.ascontiguousarray(inputs["principal_dir"][b], np.float32)
    nrm = np.ascontiguousarray(inputs["normals"][b], np.float32)
    crv = inputs["curvature"][b].astype(np.float32)
    dens = inputs["density"][b].astype(np.float32)
    lin = inputs["linearity"][b].astype(np.float32)
    qkv_w = inputs["qkv_w"].astype(np.float32)
    qkv_b = inputs["qkv_b"].astype(np.float32)
    out_w = inputs["out_w"].astype(np.float32)

    xyz = x[:, :3]
    n2 = (xyz.astype(np.float64) ** 2).sum(-1).astype(np.float32)
    cr = np.cross(pdir, nrm)
    side = cr / (np.linalg.norm(cr, axis=-1, keepdims=True) + 1e-8)
    rowdot = (xyz * pdir).sum(-1)
    rowsidedot = (xyz * side).sum(-1)

    xhi, xlo = _split_hi_lo(xyz)
    n2hi, n2lo = _split_hi_lo(n2)
    shi, slo = _split_hi_lo(side)
    rdhi, rdlo = _split_hi_lo(rowdot)
    rshi, rslo = _split_hi_lo(rowsidedot)

    ci = crv[i0 : i0 + CH]
    di = dens[i0 : i0 + CH]
    li = lin[i0 : i0 + CH]
    s_i = np.sqrt(1.0 - 0.25 * (1.0 - li)).astype(np.float32)

    grid, FI, GJ = _get_exp_basis()
    gj = np.stack([np.interp(crv, grid, GJ[:, k]) for k in range(R_EXP)])
    fi = np.stack([np.interp(ci, grid, FI[:, k]) for k in range(R_EXP)])
    gfac = (GAMMA / FOLD) * dens  # j-side factor

    # keys are rotated so this core's queries sit at columns 0:CH
    perm = (np.arange(N) + i0) % N

    # GEO rows (fp16): 0-2 xhi_j, 3-5 xlo_j, 6 n2hi, 7 n2lo, 8 ones,
    # 13-15 xhi dup, 16 ones dup, 17.. expansion g-side
    geo = np.zeros((128, N), np.float32)
    geo[0:3] = xhi.T
    geo[3:6] = xlo.T
    geo[6] = n2hi
    geo[7] = n2lo
    geo[8] = 1.0
    geo[13:16] = xhi.T
    geo[16] = 1.0
    for k in range(R_EXP):
        geo[17 + 3 * k : 20 + 3 * k] = (gj[k] * gfac)[None, :] * nrm.T
    geo = geo[:, perm]

    # rhs m0 (f32): dp' = s_i * (rowdot_i - x_j . pdir_i)
    phi, plo = _split_hi_lo(pdir[i0 : i0 + CH])
    rhsm0 = np.zeros((128, CH), np.float32)
    rhsm0[0:3] = -phi.T * s_i
    rhsm0[3:6] = -phi.T * s_i
    rhsm0[13:16] = -plo.T * s_i
    rhsm0[8] = rdhi[i0 : i0 + CH] * s_i
    rhsm0[16] = rdlo[i0 : i0 + CH] * s_i

    rhsf = np.zeros((128, 4, CH), np.float32)
    # m-slot 0: -d2 (exact negation)
    xhic = xhi[i0 : i0 + CH]
    xloc = xlo[i0 : i0 + CH]
    rhsf[0:3, 0] = 2.0 * xhic.T
    rhsf[3:6, 0] = 2.0 * xhic.T
    rhsf[13:16, 0] = 2.0 * xloc.T
    rhsf[6, 0] = -1.0
    rhsf[7, 0] = -1.0
    rhsf[8, 0] = -n2hi[i0 : i0 + CH]
    rhsf[16, 0] = -n2lo[i0 : i0 + CH]
    # m-slot 1: expansion f-side
    nic = nrm[i0 : i0 + CH]
    for k in range(R_EXP):
        rhsf[17 + 3 * k : 20 + 3 * k, 1] = fi[k][None, :] * nic.T
    # m-slot 3 row 0: E_i for the gate
    rhsf[0, 3] = HALF_W * (0.5 + di)
    # m-slot 2: lateral = rowsidedot_i - x_j . side_i
    sh, sl = shi[i0 : i0 + CH], slo[i0 : i0 + CH]
    rhsf[0:3, 2] = -sh.T
    rhsf[3:6, 2] = -sh.T
    rhsf[13:16, 2] = -sl.T
    rhsf[8, 2] = rshi[i0 : i0 + CH]
    rhsf[16, 2] = rslo[i0 : i0 + CH]
    xT = np.ascontiguousarray(x.T)[:, perm]
    f16 = np.float16

    blob16 = np.zeros((128, B16C), f16)
    blob16[:, C_I50 : C_I50 + D] = (FOLD * np.eye(D, dtype=np.float32)).astype(
        f16
    )
    blob16[:, C_WV : C_WV + D] = qkv_w[:, 2 * D : 3 * D].astype(f16)
    wqh = qkv_w[:, 0:D].reshape(D, H, HD).astype(np.float64)
    wkh = qkv_w[:, D : 2 * D].reshape(D, H, HD).astype(np.float64)
    for h in range(H):
        m_h = SC * (wqh[:, h, :] @ wkh[:, h, :].T)  # [D, D]
        blob16[:, C_WK + h * D : C_WK + (h + 1) * D] = m_h.astype(f16)
    wo_a = out_w.reshape(H, HD, D).transpose(1, 0, 2)
    blob16[0:HD, C_WO : C_WO + H * D] = wo_a.reshape(HD, H * D).astype(f16)
    blob16[0:1, C_ONES : C_ONES + D] = 1.0
    # E4 block-ones for the finish bc broadcast
    for r in range(H):
        blob16[r, C_E4 + r * HD : C_E4 + (r + 1) * HD] = 1.0

    blob32 = np.zeros((128, B32C), np.float32)
    blob32[:, 0:CH] = rhsm0
    blob32[0:HD, 512:516] = (qkv_b[0:D] * SC).reshape(H, HD).T
    blob32[0:HD, 516:520] = qkv_b[D : 2 * D].reshape(H, HD).T
    blob32[0:HD, 520:524] = qkv_b[2 * D : 3 * D].reshape(H, HD).T
    blob32[0:1, 524 : 524 + D] = inputs["out_b"].astype(np.float32)[None, :]

    return {
        "blob16": blob16,
        "blob32": blob32,
        "xt": xT.astype(f16),
        "geo": geo.astype(f16),
        "rhs": rhsf.astype(f16),
    }


def _run(inputs, trace=False):
    has_bk = bool(np.any(inputs["qkv_b"][D : 2 * D]))
    has_bv = bool(np.any(inputs["qkv_b"][2 * D : 3 * D]))
    key = ("nc", has_bk, has_bv)
    if key not in _cache:
        _cache[key] = _build_program(has_bk, has_bv)
    nc = _cache[key]
    in_maps = [_prep_core_inputs(inputs, c) for c in range(NCORES)]
    res = run_bass_kernel_spmd(nc, in_maps, core_ids=list(range(NCORES)), trace=trace)
    full = np.empty((B, N, D), np.float32)
    for c in range(NCORES):
        b, ch = c // 4, c % 4
        o = res.results[c]["out"]  # [128, NSUB, D]
        full[b, ch * CH : (ch + 1) * CH, :] = o.transpose(1, 0, 2).reshape(
            CH, D
        )
    return full, res


def kernel(**inputs):
    out, _ = _run(inputs)
    return out


# revision 50
# speedup vs baseline: 1.6039x; 1.0031x over previous
"""MultiHeadGeometryAttention Trainium2 kernel (v3).

Sharding: 8 cores = (B=2) x (N=2048 split into 4 query chunks of 512).
Each core computes the NxN geometry bias + side gate once for its 512
queries (shared by all 4 heads), then all-head attention in transposed
layout S^T[j, i] so the PV matmul contracts over keys on partitions.

Key structure (156 us baseline -> this kernel):
  - exp(-|crv_i - crv_j|) * normal_sim folded into the geometry matmul
    as a rank-64 separable expansion (PE contraction rows are free).
  - aniso term = (A2'/50) dp^2 - d2 with sqrt(A2'/50) folded into the
    d_par rhs; bias accumulates in ONE PSUM bank (ACT Square writes
    dp'^2, the -d2 / expansion matmuls accumulate on top); one eviction
    per tile, folded into scores via a 50*I fp16 identity matmul.
  - Gate sigmoid computed as (1 + tanh(-8*(|lat|-E)))/2 so every ACT
    function (Exp/Square/Abs/Tanh/Copy) lives in ONE table set -> no
    table reloads -> geometry and attention fully interleave per key
    tile inside 8 PSUM banks: geometry pa+pb (pa reused for lat),
    score half-groups 2x2, PV accumulators 2 (4 heads at partition
    bases 0/64).
  - Scores: one batched scalar_tensor_tensor per 2-head half (gate
    broadcast over heads), one batched exp per (jt, 4 heads) with bias
    -2 so e^s stays in fp16 range (cancels in softmax), PV pipelined
    one jt behind so PE never waits on the exp.
  - All tensors ship fp16 (hi/lo split hi parts fp16-exact) in few
    packed DMA blobs; fp16 matmuls run at full PE rate.
"""

import math

import numpy as np

import concourse.bass as bass  # noqa: F401
import concourse.mybir as mybir
import concourse.tile as tile
from concourse import bacc
from concourse.bass_utils import run_bass_kernel_spmd

# problem constants (fixed by the nn.Module config)
ALPHA0 = 1.0
BETA0 = 4.0
GAMMA = 0.5
SIGMA = 0.2
W_MIN, W_MAX = 0.05, 0.3
B, N, D, H = 2, 2048, 128, 4
HD = D // H  # 32
CH = 512  # query rows per core
NCORES = 8
NJT = N // 128  # 16 key tiles
NSUB = CH // 128  # 4 query subtiles

HALF_W = 0.5 * (W_MIN + W_MAX)  # 0.175
GATE_INV_SCALE = 1.0 / (0.25 * (W_MAX - W_MIN))  # 16.0
SC = 1.0 / math.sqrt(HD)
FOLD = 50.0  # geom tile carries bias/FOLD; folded back via FOLD*I matmul

R_EXP = 37  # rank of the exp(-|ci-cj|) separable expansion (geo rows 17..127)

LAG = 4  # attention trails geometry by this many key tiles
# geometry eviction engine per jt: True -> ACT, False -> DVE
GEOM_ACT = [False] * 16

# blob16 column offsets (keys are rotated per-core so xtc = xt[:, 0:CH])
C_I50, C_WV, C_WK, C_WO, C_ONES, C_E4 = 0, 128, 256, 768, 1280, 1408
B16C = 1536
# blob32: rhsm0 0-512, bq 512-516, bk 516-520, bv 520-524, ob row0 524-652
B32C = 652

F32 = mybir.dt.float32
F32R = mybir.dt.float32r
F16 = mybir.dt.float16
AF = mybir.ActivationFunctionType
ALU = mybir.AluOpType

_cache = {}


def _build_program(has_bk=False, has_bv=False):
    nc = bacc.Bacc(None)

    blob16 = nc.dram_tensor("blob16", [128, B16C], F16, kind="ExternalInput")
    blob32 = nc.dram_tensor("blob32", [128, B32C], F32, kind="ExternalInput")
    xt = nc.dram_tensor("xt", [D, N], F16, kind="ExternalInput")
    geo = nc.dram_tensor("geo", [128, N], F16, kind="ExternalInput")
    rhs = nc.dram_tensor("rhs", [128, 6, CH], F16, kind="ExternalInput")
    out = nc.dram_tensor("out", [128, NSUB, D], F32, kind="ExternalOutput")

    with tile.TileContext(nc) as tc, nc.allow_low_precision(
        reason="fp16 operands and f32r rounding are intentional"
    ):
        with (
            tc.tile_pool(name="const", bufs=1) as const,
            tc.tile_pool(name="tmp", bufs=1) as tmp,
            tc.tile_pool(name="s2p", bufs=3) as s2p,
            tc.tile_pool(name="ep", bufs=3) as ep,
            tc.tile_pool(name="gprep", bufs=2) as gprep,
            tc.tile_pool(name="latp", bufs=2) as latp,
        ):
            geo_sb = const.tile([128, N], F16, tag="geo", name="geo_sb")
            rhs_sb = const.tile([128, 6, CH], F16, tag="rhs", name="rhs_sb")
            b16 = const.tile([128, B16C], F16, tag="b16", name="b16")
            xt_sb = const.tile([D, N], F16, tag="xt", name="xt_sb")
            b32 = const.tile([128, B32C], F32, tag="b32", name="b32")
            nc.sync.dma_start(geo_sb[:, 0 : N // 2], geo[:, 0 : N // 2])
            nc.sync.dma_start(rhs_sb[:, 4:6, :], rhs[:, 4:6, :])
            nc.sync.dma_start(rhs_sb[:, 0:4, :], rhs[:, 0:4, :])
            nc.sync.dma_start(geo_sb[:, N // 2 : N], geo[:, N // 2 : N])
            nc.sync.dma_start(b16, blob16[...])
            nc.sync.dma_start(xt_sb[:, 0 : N // 2], xt[:, 0 : N // 2])
            nc.sync.dma_start(xt_sb[:, N // 2 : N], xt[:, N // 2 : N])
            nc.sync.dma_start(b32, blob32[...])

            xtc_sb = xt_sb[:, 0:CH]
            i50_sb = b16[:, C_I50 : C_I50 + D]
            wv_sb = b16[:, C_WV : C_WV + D]
            ones_sb = b16[0:1, C_ONES : C_ONES + D]

            ob_sb = const.tile([1, D], F16, tag="ob", name="ob_sb")
            nc.gpsimd.tensor_copy(ob_sb, b32[0:1, 524 : 524 + D])

            nb2 = const.tile([128, 1], F32, tag="nb2", name="nb2")
            nc.gpsimd.memset(nb2, -2.0)

            qk4 = const.tile([D, H, CH], F16, tag="qk4", name="qk4")
            V = const.tile([128, NJT, H, HD + 1], F16, tag="V", name="V")
            E16 = const.tile([128, CH], F16, tag="E16", name="E16")
            geom16 = const.tile([128, NJT, CH], F16, tag="geom16", name="geom16")
            gate16 = const.tile([128, NJT, CH], F16, tag="gate16", name="gate16")
            headcat = const.tile([HD, H, CH], F16, tag="headcat", name="headcat")

            nc.gpsimd.memset(V[:, :, :, HD : HD + 1], 1.0)

            # ---- main loop: geometry and attention interleaved ----
            with (
                tc.tile_pool(name="a_pv", bufs=1, space="PSUM") as a_pv,
            ):
                pv0 = a_pv.tile([128, CH], F32, tag="pv0", name="pv0")
                pv1 = a_pv.tile([128, CH], F32, tag="pv1", name="pv1")
                pvb = [pv0, pv1]
                prev = [None]

                with (
                    tc.tile_pool(name="g_pa", bufs=1, space="PSUM") as g_pa,
                    tc.tile_pool(name="g_pb", bufs=1, space="PSUM") as g_pb,
                    tc.tile_pool(name="a_sg", bufs=2, space="PSUM") as a_sg,
                ):
                    gpre = [None]

                    # qk4_h = M_h^T @ x_chunk with M_h = SC * Wq_h Wk_h^T
                    # host-folded (kills the q->qT->k chain on device)
                    for h in range(2):
                        sgk = a_sg.tile([128, 2, CH], F32, tag="sg",
                                        name="sgk")
                        for hh in range(2):
                            nc.tensor.matmul(
                                sgk[:, hh, :],
                                lhsT=b16[:, C_WK + (2 * h + hh) * D
                                         : C_WK + (2 * h + hh + 1) * D],
                                rhs=xtc_sb,
                                start=True,
                                stop=True,
                            )
                        nc.scalar.copy(qk4[:, 2 * h : 2 * h + 2, :], sgk)
                    sge = a_sg.tile([128, 2, CH], F32, tag="sg", name="sge")
                    nc.tensor.matmul(
                        sge[:, 0, :], lhsT=ones_sb, rhs=rhs_sb[0:1, 3, :],
                        start=True, stop=True,
                    )
                    nc.scalar.copy(E16, sge[:, 0, :])

                    def emit_g(jt):
                        j0 = jt * 128
                        lhs = geo_sb[:, j0 : j0 + 128]
                        pa1 = g_pa.tile([128, CH], F32, tag="pa", name="pa1")
                        nc.tensor.matmul(pa1, lhsT=lhs, rhs=rhs_sb[:, 4, :],
                                         start=True, stop=False)
                        nc.tensor.matmul(pa1, lhsT=lhs, rhs=rhs_sb[:, 5, :],
                                         start=False, stop=True,
                                         skip_group_check=True)
                        pb = g_pb.tile([128, CH], F32, tag="pb", name="pb")
                        nc.scalar.activation(pb, pa1, AF.Square)
                        pa2 = g_pa.tile([128, CH], F32, tag="pa", name="pa2")
                        nc.tensor.matmul(pa2, lhsT=lhs, rhs=rhs_sb[:, 2, :],
                                         start=True, stop=True)
                        nc.tensor.matmul(
                            pb, lhsT=lhs, rhs=rhs_sb[:, 0, :], start=False,
                            stop=False, skip_group_check=True,
                        )
                        nc.tensor.matmul(
                            pb, lhsT=lhs, rhs=rhs_sb[:, 1, :], start=False,
                            stop=True, skip_group_check=True,
                        )
                        if GEOM_ACT[jt]:
                            nc.scalar.copy(geom16[:, jt, :], pb)
                        else:
                            nc.vector.tensor_copy(geom16[:, jt, :], pb)
                        lata = latp.tile([128, CH], F16, tag="lata",
                                         name="lata")
                        nc.scalar.activation(lata, pa2, AF.Abs)
                        if jt % 2 == 0:
                            gpre[0] = gprep.tile([128, 2, CH], F16, tag="gp",
                                                 name="gpre")
                        nc.vector.tensor_tensor(
                            gpre[0][:, jt % 2, :], lata, E16, ALU.subtract
                        )
                        if jt % 2 == 1:
                            g = jt // 2
                            tgrp = latp.tile([128, 2, CH], F16, tag="tgrp",
                                             name="tgrp")
                            nc.scalar.activation(
                                tgrp, gpre[0], AF.Tanh,
                                scale=-0.5 * GATE_INV_SCALE,
                            )
                            # gate = 0.5 * (tanh + 1)
                            nc.vector.tensor_scalar(
                                gate16[:, g * 2 : g * 2 + 2, :], tgrp, 1.0,
                                0.5, ALU.add, ALU.mult,
                            )

                    def emit_a(jt):
                        s2 = s2p.tile([128, H, CH], F16, tag="s2", name="s2")
                        for half in range(2):
                            sg = a_sg.tile([128, 2, CH], F32, tag="sg",
                                           name="sg")
                            for hh in range(2):
                                h = 2 * half + hh
                                nc.tensor.matmul(
                                    sg[:, hh, :],
                                    lhsT=xt_sb[:, jt * 128 : (jt + 1) * 128],
                                    rhs=qk4[:, h, :],
                                    start=True,
                                    stop=False,
                                )
                                nc.tensor.matmul(
                                    sg[:, hh, :],
                                    lhsT=i50_sb,
                                    rhs=geom16[:, jt, :],
                                    start=False,
                                    stop=True,
                                    skip_group_check=True,
                                )
                            nc.vector.scalar_tensor_tensor(
                                s2[:, 2 * half : 2 * half + 2, :],
                                sg,
                                0.0,
                                gate16[:, jt : jt + 1, :].to_broadcast(
                                    [128, 2, CH]
                                ),
                                ALU.bypass,
                                ALU.mult,
                            )
                        e = ep.tile([128, H, CH], F16, tag="e", name="e")
                        nc.scalar.activation(e, s2, AF.Exp, bias=nb2)
                        if prev[0] is not None:
                            emit_pv(*prev[0])
                        prev[0] = (jt, e)

                    def emit_pv(jt, e):
                        for h in range(H):
                            b0 = (h % 2) * 64
                            nc.tensor.matmul(
                                pvb[h // 2][b0 : b0 + HD + 1, :],
                                lhsT=V[:, jt, h, :],
                                rhs=e[:, h, :],
                                start=(jt == 0),
                                stop=(jt == NJT - 1),
                                skip_group_check=True,
                            )

                    def emit_v(vj):
                        sg = a_sg.tile([128, 2, CH], F32, tag="sg", name="sgv")
                        nc.tensor.matmul(
                            sg[:, 0, 0:D],
                            lhsT=xt_sb[:, vj * 128 : (vj + 1) * 128],
                            rhs=wv_sb,
                            start=True,
                            stop=True,
                        )
                        nc.vector.tensor_copy(
                            out=V[:, vj, :, 0:HD],
                            in_=sg[:, 0, 0:D].rearrange("p (h d) -> p h d",
                                                        h=H),
                        )

                    vper = (NJT + LAG - 1) // LAG  # V-projections per G step
                    for jt in range(NJT):
                        emit_g(jt)
                        if jt < LAG:
                            for vj in range(jt * vper,
                                            min((jt + 1) * vper, NJT)):
                                emit_v(vj)
                        else:
                            emit_a(jt - LAG)
                    for jt in range(NJT - LAG, NJT):
                        emit_a(jt)
                    emit_pv(*prev[0])

                # ---- finish: normalize, concat heads, project ----
                with tc.tile_pool(name="f_ps", bufs=2, space="PSUM") as f_ps:
                    recips = []
                    for h in range(H):
                        b0 = (h % 2) * 64
                        recip = tmp.tile([1, CH], F16, tag="recip",
                                         name="recip", bufs=4)
                        nc.vector.reciprocal(
                            recip, pvb[h // 2][b0 + HD : b0 + HD + 1, :]
                        )
                        recips.append(recip)
                    bc16s = []
                    for pair in range(2):
                        bc_ps = f_ps.tile([2 * HD, CH], F32, tag="bc",
                                          name="bc_ps")
                        for hh in range(2):
                            nc.tensor.matmul(
                                bc_ps[hh * HD : (hh + 1) * HD, :],
                                lhsT=ones_sb[0:1, 0:HD],
                                rhs=recips[2 * pair + hh],
                                start=True,
                                stop=True,
                            )
                        bc16 = tmp.tile([2 * HD, CH], F16, tag="bc16",
                                        name="bc16", bufs=2)
                        nc.scalar.copy(bc16, bc_ps)
                        bc16s.append(bc16)
                    for h in range(H):
                        b0 = (h % 2) * 64
                        nc.vector.scalar_tensor_tensor(
                            headcat[:, h, :],
                            pvb[h // 2][b0 : b0 + HD, :],
                            0.0,
                            bc16s[h // 2][(h % 2) * HD : (h % 2 + 1) * HD, :],
                            ALU.bypass,
                            ALU.mult,
                        )
                        if has_bv:
                            nc.scalar.activation(
                                headcat[:, h, :], headcat[:, h, :],
                                AF.Identity,
                                bias=b32[0:HD, 520 + h : 521 + h],
                            )

                    f_all = tmp.tile([128, NSUB, D], F32, tag="fall",
                                     name="f_all")
                    for s in range(NSUB):
                        fps = f_ps.tile([128, D], F32, tag="f", name="fps")
                        for h in range(H):
                            nc.tensor.matmul(
                                fps,
                                lhsT=headcat[:, h, s * 128 : (s + 1) * 128],
                                rhs=b16[0:HD, C_WO + h * D : C_WO + (h + 1) * D],
                                start=(h == 0),
                                stop=False,
                            )
                        nc.tensor.matmul(
                            fps, lhsT=ones_sb, rhs=ob_sb, start=False,
                            stop=True,
                        )
                        if s % 2:
                            nc.scalar.copy(f_all[:, s, :], fps)
                        else:
                            nc.vector.tensor_copy(f_all[:, s, :], fps)
                    nc.sync.dma_start(out[...], f_all)

    nc.finalize()
    return nc


def _split_hi_lo(v):
    """Split fp32 array into an fp16-exact hi part and the fp32 residual."""
    v = np.asarray(v, np.float32)
    hi = v.astype(np.float16).astype(np.float32)
    lo = (v.astype(np.float64) - hi).astype(np.float32)
    return hi, lo


_exp_basis = None


def _get_exp_basis():
    """Separable rank-R_EXP expansion of exp(-|a-b|) on [0,1]^2."""
    global _exp_basis
    if _exp_basis is None:
        g = np.linspace(0.0, 1.0, 2048)
        K = np.exp(-np.abs(g[:, None] - g[None, :]))
        U, s, Vt = np.linalg.svd(K)
        r = R_EXP
        FI = U[:, :r] * np.sqrt(s[:r])
        GJ = Vt[:r].T * np.sqrt(s[:r])
        _exp_basis = (g, FI, GJ)
    return _exp_basis


def _prep_core_inputs(inputs, core):
    b, ch = core // 4, core % 4
    i0 = ch * CH
    x = np.ascontiguousarray(inputs["x"][b], np.float32)  # [N, D]
    pdir = np.ascontiguousarray(inputs["principal_dir"][b], np.float32)
    nrm = np.ascontiguousarray(inputs["normals"][b], np.float32)
    crv = inputs["curvature"][b].astype(np.float32)
    dens = inputs["density"][b].astype(np.float32)
    lin = inputs["linearity"][b].astype(np.float32)
    qkv_w = inputs["qkv_w"].astype(np.float32)
    qkv_b = inputs["qkv_b"].astype(np.float32)
    out_w = inputs["out_w"].astype(np.float32)

    xyz = x[:, :3]
    n2 = (xyz.astype(np.float64) ** 2).sum(-1).astype(np.float32)
    cr = np.cross(pdir, nrm)
    side = cr / (np.linalg.norm(cr, axis=-1, keepdims=True) + 1e-8)
    rowdot = (xyz * pdir).sum(-1)
    rowsidedot = (xyz * side).sum(-1)

    xhi, xlo = _split_hi_lo(xyz)
    n2hi, n2lo = _split_hi_lo(n2)
    shi, slo = _split_hi_lo(side)
    rdhi, rdlo = _split_hi_lo(rowdot)
    rshi, rslo = _split_hi_lo(rowsidedot)

    ci = crv[i0 : i0 + CH]
    di = dens[i0 : i0 + CH]
    li = lin[i0 : i0 + CH]
    s_i = np.sqrt(1.0 - 0.25 * (1.0 - li)).astype(np.float32)

    grid, FI, GJ = _get_exp_basis()
    gj = np.stack([np.interp(crv, grid, GJ[:, k]) for k in range(R_EXP)])
    fi = np.stack([np.interp(ci, grid, FI[:, k]) for k in range(R_EXP)])
    gfac = (GAMMA / FOLD) * dens  # j-side factor

    # keys are rotated so this core's queries sit at columns 0:CH
    perm = (np.arange(N) + i0) % N

    # GEO rows (fp16): 0-2 xhi_j, 3-5 xlo_j, 6 n2hi, 7 n2lo, 8 ones,
    # 13-15 xhi dup, 16 ones dup, 17.. expansion g-side
    geo = np.zeros((128, N), np.float32)
    geo[0:3] = xhi.T
    geo[3:6] = xlo.T
    geo[6] = n2hi
    geo[7] = n2lo
    geo[8] = 1.0
    geo[13:16] = xhi.T
    geo[16] = 1.0
    for k in range(R_EXP):
        geo[17 + 3 * k : 20 + 3 * k] = (gj[k] * gfac)[None, :] * nrm.T
    geo = geo[:, perm]

    # rhs m0 (f32): dp' = s_i * (rowdot_i - x_j . pdir_i)
    phi, plo = _split_hi_lo(pdir[i0 : i0 + CH])
    rhsm0 = np.zeros((128, CH), np.float32)
    rhsm0[0:3] = -phi.T * s_i
    rhsm0[3:6] = -phi.T * s_i
    rhsm0[13:16] = -plo.T * s_i
    rhsm0[8] = rdhi[i0 : i0 + CH] * s_i
    rhsm0[16] = rdlo[i0 : i0 + CH] * s_i

    rhsf = np.zeros((128, 6, CH), np.float32)
    # m-slot 0: -d2 (exact negation)
    xhic = xhi[i0 : i0 + CH]
    xloc = xlo[i0 : i0 + CH]
    rhsf[0:3, 0] = 2.0 * xhic.T
    rhsf[3:6, 0] = 2.0 * xhic.T
    rhsf[13:16, 0] = 2.0 * xloc.T
    rhsf[6, 0] = -1.0
    rhsf[7, 0] = -1.0
    rhsf[8, 0] = -n2hi[i0 : i0 + CH]
    rhsf[16, 0] = -n2lo[i0 : i0 + CH]
    # m-slot 1: expansion f-side
    nic = nrm[i0 : i0 + CH]
    for k in range(R_EXP):
        rhsf[17 + 3 * k : 20 + 3 * k, 1] = fi[k][None, :] * nic.T
    # m-slot 3 row 0: E_i for the gate
    rhsf[0, 3] = HALF_W * (0.5 + di)
    # m-slots 4/5: dp' rhs as fp16 hi + lo
    m0hi = rhsm0.astype(np.float16).astype(np.float32)
    rhsf[:, 4] = m0hi
    rhsf[:, 5] = rhsm0 - m0hi
    # m-slot 2: lateral = rowsidedot_i - x_j . side_i
    sh, sl = shi[i0 : i0 + CH], slo[i0 : i0 + CH]
    rhsf[0:3, 2] = -sh.T
    rhsf[3:6, 2] = -sh.T
    rhsf[13:16, 2] = -sl.T
    rhsf[8, 2] = rshi[i0 : i0 + CH]
    rhsf[16, 2] = rslo[i0 : i0 + CH]
    xT = np.ascontiguousarray(x.T)[:, perm]
    f16 = np.float16

    blob16 = np.zeros((128, B16C), f16)
    blob16[:, C_I50 : C_I50 + D] = (FOLD * np.eye(D, dtype=np.float32)).astype(
        f16
    )
    blob16[:, C_WV : C_WV + D] = qkv_w[:, 2 * D : 3 * D].astype(f16)
    wqh = qkv_w[:, 0:D].reshape(D, H, HD).astype(np.float64)
    wkh = qkv_w[:, D : 2 * D].reshape(D, H, HD).astype(np.float64)
    for h in range(H):
        m_h = SC * (wqh[:, h, :] @ wkh[:, h, :].T)  # [D, D]
        blob16[:, C_WK + h * D : C_WK + (h + 1) * D] = m_h.astype(f16)
    wo_a = out_w.reshape(H, HD, D).transpose(1, 0, 2)
    blob16[0:HD, C_WO : C_WO + H * D] = wo_a.reshape(HD, H * D).astype(f16)
    blob16[0:1, C_ONES : C_ONES + D] = 1.0
    # E4 block-ones for the finish bc broadcast
    for r in range(H):
        blob16[r, C_E4 + r * HD : C_E4 + (r + 1) * HD] = 1.0

    blob32 = np.zeros((128, B32C), np.float32)
    blob32[0:HD, 512:516] = (qkv_b[0:D] * SC).reshape(H, HD).T
    blob32[0:HD, 516:520] = qkv_b[D : 2 * D].reshape(H, HD).T
    blob32[0:HD, 520:524] = qkv_b[2 * D : 3 * D].reshape(H, HD).T
    blob32[0:1, 524 : 524 + D] = inputs["out_b"].astype(np.float32)[None, :]

    return {
        "blob16": blob16,
        "blob32": blob32,
        "xt": xT.astype(f16),
        "geo": geo.astype(f16),
        "rhs": rhsf.astype(f16),
    }


def _run(inputs, trace=False):
    has_bk = bool(np.any(inputs["qkv_b"][D : 2 * D]))
    has_bv = bool(np.any(inputs["qkv_b"][2 * D : 3 * D]))
    key = ("nc", has_bk, has_bv)
    if key not in _cache:
        _cache[key] = _build_program(has_bk, has_bv)
    nc = _cache[key]
    in_maps = [_prep_core_inputs(inputs, c) for c in range(NCORES)]
    res = run_bass_kernel_spmd(nc, in_maps, core_ids=list(range(NCORES)), trace=trace)
    full = np.empty((B, N, D), np.float32)
    for c in range(NCORES):
        b, ch = c // 4, c % 4
        o = res.results[c]["out"]  # [128, NSUB, D]
        full[b, ch * CH : (ch + 1) * CH, :] = o.transpose(1, 0, 2).reshape(
            CH, D
        )
    return full, res


def kernel(**inputs):
    out, _ = _run(inputs)
    return out


# revision 54
# speedup vs baseline: 1.6078x; 1.0024x over previous
"""MultiHeadGeometryAttention Trainium2 kernel (v3).

Sharding: 8 cores = (B=2) x (N=2048 split into 4 query chunks of 512).
Each core computes the NxN geometry bias + side gate once for its 512
queries (shared by all 4 heads), then all-head attention in transposed
layout S^T[j, i] so the PV matmul contracts over keys on partitions.

Key structure (156 us baseline -> this kernel):
  - exp(-|crv_i - crv_j|) * normal_sim folded into the geometry matmul
    as a rank-64 separable expansion (PE contraction rows are free).
  - aniso term = (A2'/50) dp^2 - d2 with sqrt(A2'/50) folded into the
    d_par rhs; bias accumulates in ONE PSUM bank (ACT Square writes
    dp'^2, the -d2 / expansion matmuls accumulate on top); one eviction
    per tile, folded into scores via a 50*I fp16 identity matmul.
  - Gate sigmoid computed as (1 + tanh(-8*(|lat|-E)))/2 so every ACT
    function (Exp/Square/Abs/Tanh/Copy) lives in ONE table set -> no
    table reloads -> geometry and attention fully interleave per key
    tile inside 8 PSUM banks: geometry pa+pb (pa reused for lat),
    score half-groups 2x2, PV accumulators 2 (4 heads at partition
    bases 0/64).
  - Scores: one batched scalar_tensor_tensor per 2-head half (gate
    broadcast over heads), one batched exp per (jt, 4 heads) with bias
    -2 so e^s stays in fp16 range (cancels in softmax), PV pipelined
    one jt behind so PE never waits on the exp.
  - All tensors ship fp16 (hi/lo split hi parts fp16-exact) in few
    packed DMA blobs; fp16 matmuls run at full PE rate.
"""

import math

import numpy as np

import concourse.bass as bass  # noqa: F401
import concourse.mybir as mybir
import concourse.tile as tile
from concourse import bacc
from concourse.bass_utils import run_bass_kernel_spmd

# problem constants (fixed by the nn.Module config)
ALPHA0 = 1.0
BETA0 = 4.0
GAMMA = 0.5
SIGMA = 0.2
W_MIN, W_MAX = 0.05, 0.3
B, N, D, H = 2, 2048, 128, 4
HD = D // H  # 32
CH = 512  # query rows per core
NCORES = 8
NJT = N // 128  # 16 key tiles
NSUB = CH // 128  # 4 query subtiles

HALF_W = 0.5 * (W_MIN + W_MAX)  # 0.175
GATE_INV_SCALE = 1.0 / (0.25 * (W_MAX - W_MIN))  # 16.0
SC = 1.0 / math.sqrt(HD)
FOLD = 50.0  # geom tile carries bias/FOLD; folded back via FOLD*I matmul

R_EXP = 37  # rank of the exp(-|ci-cj|) separable expansion (geo rows 17..127)

LAG = 4  # attention trails geometry by this many key tiles
# geometry eviction engine per jt: True -> ACT, False -> DVE
GEOM_ACT = [False] * 16

# blob16 column offsets (keys are rotated per-core so xtc = xt[:, 0:CH])
C_I50, C_WV, C_WK, C_WO, C_ONES, C_E4 = 0, 128, 256, 768, 1280, 1408
B16C = 1536
# blob32: rhsm0 0-512, bq 512-516, bk 516-520, bv 520-524, ob row0 524-652
B32C = 652

F32 = mybir.dt.float32
F32R = mybir.dt.float32r
F16 = mybir.dt.float16
AF = mybir.ActivationFunctionType
ALU = mybir.AluOpType

_cache = {}


def _build_program(has_bk=False, has_bv=False):
    nc = bacc.Bacc(None)

    blob16 = nc.dram_tensor("blob16", [128, B16C], F16, kind="ExternalInput")
    blob32 = nc.dram_tensor("blob32", [128, B32C], F32, kind="ExternalInput")
    xt = nc.dram_tensor("xt", [D, N], F16, kind="ExternalInput")
    geo = nc.dram_tensor("geo", [128, N], F16, kind="ExternalInput")
    rhs = nc.dram_tensor("rhs", [128, 6, CH], F16, kind="ExternalInput")
    out = nc.dram_tensor("out", [128, NSUB, D], F32, kind="ExternalOutput")

    with tile.TileContext(nc) as tc, nc.allow_low_precision(
        reason="fp16 operands and f32r rounding are intentional"
    ):
        with (
            tc.tile_pool(name="const", bufs=1) as const,
            tc.tile_pool(name="tmp", bufs=1) as tmp,
            tc.tile_pool(name="s2p", bufs=4) as s2p,
            tc.tile_pool(name="ep", bufs=4) as ep,
            tc.tile_pool(name="gprep", bufs=2) as gprep,
            tc.tile_pool(name="latp", bufs=2) as latp,
        ):
            geo_sb = const.tile([128, N], F16, tag="geo", name="geo_sb")
            rhs_sb = const.tile([128, 6, CH], F16, tag="rhs", name="rhs_sb")
            b16 = const.tile([128, B16C], F16, tag="b16", name="b16")
            xt_sb = const.tile([D, N], F16, tag="xt", name="xt_sb")
            b32 = const.tile([128, B32C], F32, tag="b32", name="b32")
            nc.sync.dma_start(geo_sb[:, 0 : N // 2], geo[:, 0 : N // 2])
            nc.sync.dma_start(rhs_sb[:, 4:6, :], rhs[:, 4:6, :])
            nc.sync.dma_start(rhs_sb[:, 0:4, :], rhs[:, 0:4, :])
            nc.sync.dma_start(geo_sb[:, N // 2 : N], geo[:, N // 2 : N])
            nc.sync.dma_start(b16, blob16[...])
            nc.sync.dma_start(xt_sb[:, 0 : N // 2], xt[:, 0 : N // 2])
            nc.sync.dma_start(xt_sb[:, N // 2 : N], xt[:, N // 2 : N])
            nc.sync.dma_start(b32, blob32[...])

            xtc_sb = xt_sb[:, 0:CH]
            i50_sb = b16[:, C_I50 : C_I50 + D]
            wv_sb = b16[:, C_WV : C_WV + D]
            ones_sb = b16[0:1, C_ONES : C_ONES + D]

            ob_sb = const.tile([1, D], F16, tag="ob", name="ob_sb")
            nc.gpsimd.tensor_copy(ob_sb, b32[0:1, 524 : 524 + D])

            nb2 = const.tile([128, 1], F32, tag="nb2", name="nb2")
            nc.gpsimd.memset(nb2, -2.0)

            qk4 = const.tile([D, H, CH], F16, tag="qk4", name="qk4")
            V = const.tile([128, NJT, H, HD + 1], F16, tag="V", name="V")
            E16 = const.tile([128, CH], F16, tag="E16", name="E16")
            geom16 = const.tile([128, NJT, CH], F16, tag="geom16", name="geom16")
            gate16 = const.tile([128, NJT, CH], F16, tag="gate16", name="gate16")
            headcat = const.tile([HD, H, CH], F16, tag="headcat", name="headcat")

            nc.gpsimd.memset(V[:, :, :, HD : HD + 1], 1.0)

            # ---- main loop: geometry and attention interleaved ----
            with (
                tc.tile_pool(name="a_pv", bufs=1, space="PSUM") as a_pv,
            ):
                pv0 = a_pv.tile([128, CH], F32, tag="pv0", name="pv0")
                pv1 = a_pv.tile([128, CH], F32, tag="pv1", name="pv1")
                pvb = [pv0, pv1]
                prev = [None]

                with (
                    tc.tile_pool(name="g_pa", bufs=1, space="PSUM") as g_pa,
                    tc.tile_pool(name="g_pb", bufs=1, space="PSUM") as g_pb,
                    tc.tile_pool(name="a_sg", bufs=2, space="PSUM") as a_sg,
                ):
                    gpre = [None]

                    # qk4_h = M_h^T @ x_chunk with M_h = SC * Wq_h Wk_h^T
                    # host-folded (kills the q->qT->k chain on device)
                    for h in range(2):
                        sgk = a_sg.tile([128, 2, CH], F32, tag="sg",
                                        name="sgk")
                        for hh in range(2):
                            nc.tensor.matmul(
                                sgk[:, hh, :],
                                lhsT=b16[:, C_WK + (2 * h + hh) * D
                                         : C_WK + (2 * h + hh + 1) * D],
                                rhs=xtc_sb,
                                start=True,
                                stop=True,
                            )
                        nc.scalar.copy(qk4[:, 2 * h : 2 * h + 2, :], sgk)
                    sge = a_sg.tile([128, 2, CH], F32, tag="sg", name="sge")
                    nc.tensor.matmul(
                        sge[:, 0, :], lhsT=ones_sb, rhs=rhs_sb[0:1, 3, :],
                        start=True, stop=True,
                    )
                    nc.scalar.copy(E16, sge[:, 0, :])

                    def emit_g(jt):
                        j0 = jt * 128
                        lhs = geo_sb[:, j0 : j0 + 128]
                        pa1 = g_pa.tile([128, CH], F32, tag="pa", name="pa1")
                        nc.tensor.matmul(pa1, lhsT=lhs, rhs=rhs_sb[:, 4, :],
                                         start=True, stop=False)
                        nc.tensor.matmul(pa1, lhsT=lhs, rhs=rhs_sb[:, 5, :],
                                         start=False, stop=True,
                                         skip_group_check=True)
                        pb = g_pb.tile([128, CH], F32, tag="pb", name="pb")
                        nc.scalar.activation(pb, pa1, AF.Square)
                        pa2 = g_pa.tile([128, CH], F32, tag="pa", name="pa2")
                        nc.tensor.matmul(pa2, lhsT=lhs, rhs=rhs_sb[:, 2, :],
                                         start=True, stop=True)
                        nc.tensor.matmul(
                            pb, lhsT=lhs, rhs=rhs_sb[:, 0, :], start=False,
                            stop=False, skip_group_check=True,
                        )
                        nc.tensor.matmul(
                            pb, lhsT=lhs, rhs=rhs_sb[:, 1, :], start=False,
                            stop=True, skip_group_check=True,
                        )
                        if GEOM_ACT[jt]:
                            nc.scalar.copy(geom16[:, jt, :], pb)
                        else:
                            nc.vector.tensor_copy(geom16[:, jt, :], pb)
                        lata = latp.tile([128, CH], F16, tag="lata",
                                         name="lata")
                        nc.scalar.activation(lata, pa2, AF.Abs)
                        if jt % 2 == 0:
                            gpre[0] = gprep.tile([128, 2, CH], F16, tag="gp",
                                                 name="gpre")
                        nc.vector.tensor_tensor(
                            gpre[0][:, jt % 2, :], lata, E16, ALU.subtract
                        )
                        if jt % 2 == 1:
                            g = jt // 2
                            tgrp = latp.tile([128, 2, CH], F16, tag="tgrp",
                                             name="tgrp")
                            nc.scalar.activation(
                                tgrp, gpre[0], AF.Tanh,
                                scale=-0.5 * GATE_INV_SCALE,
                            )
                            # gate = 0.5 * (tanh + 1)
                            nc.vector.tensor_scalar(
                                gate16[:, g * 2 : g * 2 + 2, :], tgrp, 1.0,
                                0.5, ALU.add, ALU.mult,
                            )

                    def emit_a(jt):
                        s2 = s2p.tile([128, H, CH], F16, tag="s2", name="s2")
                        for half in range(2):
                            sg = a_sg.tile([128, 2, CH], F32, tag="sg",
                                           name="sg")
                            for hh in range(2):
                                h = 2 * half + hh
                                nc.tensor.matmul(
                                    sg[:, hh, :],
                                    lhsT=xt_sb[:, jt * 128 : (jt + 1) * 128],
                                    rhs=qk4[:, h, :],
                                    start=True,
                                    stop=False,
                                )
                                nc.tensor.matmul(
                                    sg[:, hh, :],
                                    lhsT=i50_sb,
                                    rhs=geom16[:, jt, :],
                                    start=False,
                                    stop=True,
                                    skip_group_check=True,
                                )
                            nc.vector.scalar_tensor_tensor(
                                s2[:, 2 * half : 2 * half + 2, :],
                                sg,
                                0.0,
                                gate16[:, jt : jt + 1, :].to_broadcast(
                                    [128, 2, CH]
                                ),
                                ALU.bypass,
                                ALU.mult,
                            )
                        e = ep.tile([128, H, CH], F16, tag="e", name="e")
                        nc.scalar.activation(e, s2, AF.Exp, bias=nb2)
                        if prev[0] is not None:
                            emit_pv(*prev[0])
                        prev[0] = (jt, e)

                    def emit_pv(jt, e):
                        for h in range(H):
                            b0 = (h % 2) * 64
                            nc.tensor.matmul(
                                pvb[h // 2][b0 : b0 + HD + 1, :],
                                lhsT=V[:, jt, h, :],
                                rhs=e[:, h, :],
                                start=(jt == 0),
                                stop=(jt == NJT - 1),
                                skip_group_check=True,
                            )

                    def emit_v(vj):
                        sg = a_sg.tile([128, 2, CH], F32, tag="sg", name="sgv")
                        nc.tensor.matmul(
                            sg[:, 0, 0:D],
                            lhsT=xt_sb[:, vj * 128 : (vj + 1) * 128],
                            rhs=wv_sb,
                            start=True,
                            stop=True,
                        )
                        nc.vector.tensor_copy(
                            out=V[:, vj, :, 0:HD],
                            in_=sg[:, 0, 0:D].rearrange("p (h d) -> p h d",
                                                        h=H),
                        )

                    vper = (NJT + LAG - 1) // LAG  # V-projections per G step
                    for jt in range(NJT):
                        emit_g(jt)
                        if jt < LAG:
                            for vj in range(jt * vper,
                                            min((jt + 1) * vper, NJT)):
                                emit_v(vj)
                        else:
                            emit_a(jt - LAG)
                    for jt in range(NJT - LAG, NJT):
                        emit_a(jt)
                    emit_pv(*prev[0])

                # ---- finish: normalize, concat heads, project ----
                with tc.tile_pool(name="f_ps", bufs=2, space="PSUM") as f_ps:
                    recips = []
                    for h in range(H):
                        b0 = (h % 2) * 64
                        recip = tmp.tile([1, CH], F16, tag="recip",
                                         name="recip", bufs=4)
                        nc.vector.reciprocal(
                            recip, pvb[h // 2][b0 + HD : b0 + HD + 1, :]
                        )
                        recips.append(recip)
                    bc16s = []
                    for pair in range(2):
                        bc_ps = f_ps.tile([2 * HD, CH], F32, tag="bc",
                                          name="bc_ps")
                        for hh in range(2):
                            nc.tensor.matmul(
                                bc_ps[hh * HD : (hh + 1) * HD, :],
                                lhsT=ones_sb[0:1, 0:HD],
                                rhs=recips[2 * pair + hh],
                                start=True,
                                stop=True,
                            )
                        bc16 = tmp.tile([2 * HD, CH], F16, tag="bc16",
                                        name="bc16", bufs=2)
                        nc.scalar.copy(bc16, bc_ps)
                        bc16s.append(bc16)
                    for h in range(H):
                        b0 = (h % 2) * 64
                        nc.vector.scalar_tensor_tensor(
                            headcat[:, h, :],
                            pvb[h // 2][b0 : b0 + HD, :],
                            0.0,
                            bc16s[h // 2][(h % 2) * HD : (h % 2 + 1) * HD, :],
                            ALU.bypass,
                            ALU.mult,
                        )
                        if has_bv:
                            nc.scalar.activation(
                                headcat[:, h, :], headcat[:, h, :],
                                AF.Identity,
                                bias=b32[0:HD, 520 + h : 521 + h],
                            )

                    f_all = tmp.tile([128, NSUB, D], F32, tag="fall",
                                     name="f_all")
                    for s in range(NSUB):
                        fps = f_ps.tile([128, D], F32, tag="f", name="fps")
                        for h in range(H):
                            nc.tensor.matmul(
                                fps,
                                lhsT=headcat[:, h, s * 128 : (s + 1) * 128],
                                rhs=b16[0:HD, C_WO + h * D : C_WO + (h + 1) * D],
                                start=(h == 0),
                                stop=False,
                            )
                        nc.tensor.matmul(
                            fps, lhsT=ones_sb, rhs=ob_sb, start=False,
                            stop=True,
                        )
                        if s % 2:
                            nc.scalar.copy(f_all[:, s, :], fps)
                        else:
                            nc.vector.tensor_copy(f_all[:, s, :], fps)
                    nc.sync.dma_start(out[...], f_all)

    nc.finalize()
    return nc


def _split_hi_lo(v):
    """Split fp32 array into an fp16-exact hi part and the fp32 residual."""
    v = np.asarray(v, np.float32)
    hi = v.astype(np.float16).astype(np.float32)
    lo = (v.astype(np.float64) - hi).astype(np.float32)
    return hi, lo


_exp_basis = None


def _get_exp_basis():
    """Separable rank-R_EXP expansion of exp(-|a-b|) on [0,1]^2."""
    global _exp_basis
    if _exp_basis is None:
        g = np.linspace(0.0, 1.0, 2048)
        K = np.exp(-np.abs(g[:, None] - g[None, :]))
        U, s, Vt = np.linalg.svd(K)
        r = R_EXP
        FI = U[:, :r] * np.sqrt(s[:r])
        GJ = Vt[:r].T * np.sqrt(s[:r])
        _exp_basis = (g, FI, GJ)
    return _exp_basis


def _prep_core_inputs(inputs, core):
    b, ch = core // 4, core % 4
    i0 = ch * CH
    x = np.ascontiguousarray(inputs["x"][b], np.float32)  # [N, D]
    pdir = np.ascontiguousarray(inputs["principal_dir"][b], np.float32)
    nrm = np.ascontiguousarray(inputs["normals"][b], np.float32)
    crv = inputs["curvature"][b].astype(np.float32)
    dens = inputs["density"][b].astype(np.float32)
    lin = inputs["linearity"][b].astype(np.float32)
    qkv_w = inputs["qkv_w"].astype(np.float32)
    qkv_b = inputs["qkv_b"].astype(np.float32)
    out_w = inputs["out_w"].astype(np.float32)

    xyz = x[:, :3]
    n2 = (xyz.astype(np.float64) ** 2).sum(-1).astype(np.float32)
    cr = np.cross(pdir, nrm)
    side = cr / (np.linalg.norm(cr, axis=-1, keepdims=True) + 1e-8)
    rowdot = (xyz * pdir).sum(-1)
    rowsidedot = (xyz * side).sum(-1)

    xhi, xlo = _split_hi_lo(xyz)
    n2hi, n2lo = _split_hi_lo(n2)
    shi, slo = _split_hi_lo(side)
    rdhi, rdlo = _split_hi_lo(rowdot)
    rshi, rslo = _split_hi_lo(rowsidedot)

    ci = crv[i0 : i0 + CH]
    di = dens[i0 : i0 + CH]
    li = lin[i0 : i0 + CH]
    s_i = np.sqrt(1.0 - 0.25 * (1.0 - li)).astype(np.float32)

    grid, FI, GJ = _get_exp_basis()
    gj = np.stack([np.interp(crv, grid, GJ[:, k]) for k in range(R_EXP)])
    fi = np.stack([np.interp(ci, grid, FI[:, k]) for k in range(R_EXP)])
    gfac = (GAMMA / FOLD) * dens  # j-side factor

    # keys are rotated so this core's queries sit at columns 0:CH
    perm = (np.arange(N) + i0) % N

    # GEO rows (fp16): 0-2 xhi_j, 3-5 xlo_j, 6 n2hi, 7 n2lo, 8 ones,
    # 13-15 xhi dup, 16 ones dup, 17.. expansion g-side
    geo = np.zeros((128, N), np.float32)
    geo[0:3] = xhi.T
    geo[3:6] = xlo.T
    geo[6] = n2hi
    geo[7] = n2lo
    geo[8] = 1.0
    geo[13:16] = xhi.T
    geo[16] = 1.0
    for k in range(R_EXP):
        geo[17 + 3 * k : 20 + 3 * k] = (gj[k] * gfac)[None, :] * nrm.T
    geo = geo[:, perm]

    # rhs m0 (f32): dp' = s_i * (rowdot_i - x_j . pdir_i)
    phi, plo = _split_hi_lo(pdir[i0 : i0 + CH])
    rhsm0 = np.zeros((128, CH), np.float32)
    rhsm0[0:3] = -phi.T * s_i
    rhsm0[3:6] = -phi.T * s_i
    rhsm0[13:16] = -plo.T * s_i
    rhsm0[8] = rdhi[i0 : i0 + CH] * s_i
    rhsm0[16] = rdlo[i0 : i0 + CH] * s_i

    rhsf = np.zeros((128, 6, CH), np.float32)
    # m-slot 0: -d2 (exact negation)
    xhic = xhi[i0 : i0 + CH]
    xloc = xlo[i0 : i0 + CH]
    rhsf[0:3, 0] = 2.0 * xhic.T
    rhsf[3:6, 0] = 2.0 * xhic.T
    rhsf[13:16, 0] = 2.0 * xloc.T
    rhsf[6, 0] = -1.0
    rhsf[7, 0] = -1.0
    rhsf[8, 0] = -n2hi[i0 : i0 + CH]
    rhsf[16, 0] = -n2lo[i0 : i0 + CH]
    # m-slot 1: expansion f-side
    nic = nrm[i0 : i0 + CH]
    for k in range(R_EXP):
        rhsf[17 + 3 * k : 20 + 3 * k, 1] = fi[k][None, :] * nic.T
    # m-slot 3 row 0: E_i for the gate
    rhsf[0, 3] = HALF_W * (0.5 + di)
    # m-slots 4/5: dp' rhs as fp16 hi + lo
    m0hi = rhsm0.astype(np.float16).astype(np.float32)
    rhsf[:, 4] = m0hi
    rhsf[:, 5] = rhsm0 - m0hi
    # m-slot 2: lateral = rowsidedot_i - x_j . side_i
    sh, sl = shi[i0 : i0 + CH], slo[i0 : i0 + CH]
    rhsf[0:3, 2] = -sh.T
    rhsf[3:6, 2] = -sh.T
    rhsf[13:16, 2] = -sl.T
    rhsf[8, 2] = rshi[i0 : i0 + CH]
    rhsf[16, 2] = rslo[i0 : i0 + CH]
    xT = np.ascontiguousarray(x.T)[:, perm]
    f16 = np.float16

    blob16 = np.zeros((128, B16C), f16)
    blob16[:, C_I50 : C_I50 + D] = (FOLD * np.eye(D, dtype=np.float32)).astype(
        f16
    )
    blob16[:, C_WV : C_WV + D] = qkv_w[:, 2 * D : 3 * D].astype(f16)
    wqh = qkv_w[:, 0:D].reshape(D, H, HD).astype(np.float64)
    wkh = qkv_w[:, D : 2 * D].reshape(D, H, HD).astype(np.float64)
    for h in range(H):
        m_h = SC * (wqh[:, h, :] @ wkh[:, h, :].T)  # [D, D]
        blob16[:, C_WK + h * D : C_WK + (h + 1) * D] = m_h.astype(f16)
    wo_a = out_w.reshape(H, HD, D).transpose(1, 0, 2)
    blob16[0:HD, C_WO : C_WO + H * D] = wo_a.reshape(HD, H * D).astype(f16)
    blob16[0:1, C_ONES : C_ONES + D] = 1.0
    # E4 block-ones for the finish bc broadcast
    for r in range(H):
        blob16[r, C_E4 + r * HD : C_E4 + (r + 1) * HD] = 1.0

    blob32 = np.zeros((128, B32C), np.float32)
    blob32[0:HD, 512:516] = (qkv_b[0:D] * SC).reshape(H, HD).T
    blob32[0:HD, 516:520] = qkv_b[D : 2 * D].reshape(H, HD).T
    blob32[0:HD, 520:524] = qkv_b[2 * D : 3 * D].reshape(H, HD).T
    blob32[0:1, 524 : 524 + D] = inputs["out_b"].astype(np.float32)[None, :]

    return {
        "blob16": blob16,
        "blob32": blob32,
        "xt": xT.astype(f16),
        "geo": geo.astype(f16),
        "rhs": rhsf.astype(f16),
    }


def _run(inputs, trace=False):
    has_bk = bool(np.any(inputs["qkv_b"][D : 2 * D]))
    has_bv = bool(np.any(inputs["qkv_b"][2 * D : 3 * D]))
    key = ("nc", has_bk, has_bv)
    if key not in _cache:
        _cache[key] = _build_program(has_bk, has_bv)
    nc = _cache[key]
    in_maps = [_prep_core_inputs(inputs, c) for c in range(NCORES)]
    res = run_bass_kernel_spmd(nc, in_maps, core_ids=list(range(NCORES)), trace=trace)
    full = np.empty((B, N, D), np.float32)
    for c in range(NCORES):
        b, ch = c // 4, c % 4
        o = res.results[c]["out"]  # [128, NSUB, D]
        full[b, ch * CH : (ch + 1) * CH, :] = o.transpose(1, 0, 2).reshape(
            CH, D
        )
    return full, res


def kernel(**inputs):
    out, _ = _run(inputs)
    return out


# revision 55
# speedup vs baseline: 1.7898x; 1.1132x over previous
"""MultiHeadGeometryAttention Trainium2 kernel (v3).

Sharding: 8 cores = (B=2) x (N=2048 split into 4 query chunks of 512).
Each core computes the NxN geometry bias + side gate once for its 512
queries (shared by all 4 heads), then all-head attention in transposed
layout S^T[j, i] so the PV matmul contracts over keys on partitions.

Key structure (156 us baseline -> this kernel):
  - exp(-|crv_i - crv_j|) * normal_sim folded into the geometry matmul
    as a rank-64 separable expansion (PE contraction rows are free).
  - aniso term = (A2'/50) dp^2 - d2 with sqrt(A2'/50) folded into the
    d_par rhs; bias accumulates in ONE PSUM bank (ACT Square writes
    dp'^2, the -d2 / expansion matmuls accumulate on top); one eviction
    per tile, folded into scores via a 50*I fp16 identity matmul.
  - Gate sigmoid computed as (1 + tanh(-8*(|lat|-E)))/2 so every ACT
    function (Exp/Square/Abs/Tanh/Copy) lives in ONE table set -> no
    table reloads -> geometry and attention fully interleave per key
    tile inside 8 PSUM banks: geometry pa+pb (pa reused for lat),
    score half-groups 2x2, PV accumulators 2 (4 heads at partition
    bases 0/64).
  - Scores: one batched scalar_tensor_tensor per 2-head half (gate
    broadcast over heads), one batched exp per (jt, 4 heads) with bias
    -2 so e^s stays in fp16 range (cancels in softmax), PV pipelined
    one jt behind so PE never waits on the exp.
  - All tensors ship fp16 (hi/lo split hi parts fp16-exact) in few
    packed DMA blobs; fp16 matmuls run at full PE rate.
"""

import math

import numpy as np

import concourse.bass as bass  # noqa: F401
import concourse.mybir as mybir
import concourse.tile as tile
from concourse import bacc
from concourse.bass_utils import run_bass_kernel_spmd

# problem constants (fixed by the nn.Module config)
ALPHA0 = 1.0
BETA0 = 4.0
GAMMA = 0.5
SIGMA = 0.2
W_MIN, W_MAX = 0.05, 0.3
B, N, D, H = 2, 2048, 128, 4
HD = D // H  # 32
CH = 512  # query rows per core
NCORES = 8
NJT = N // 128  # 16 key tiles
NSUB = CH // 128  # 4 query subtiles

HALF_W = 0.5 * (W_MIN + W_MAX)  # 0.175
GATE_INV_SCALE = 1.0 / (0.25 * (W_MAX - W_MIN))  # 16.0
SC = 1.0 / math.sqrt(HD)
FOLD = 50.0  # geom tile carries bias/FOLD; folded back via FOLD*I matmul

R_EXP = 37  # rank of the exp(-|ci-cj|) separable expansion (geo rows 17..127)

LAG = 4  # attention trails geometry by this many key tiles
# geometry eviction engine per jt: True -> ACT, False -> DVE
GEOM_ACT = [False] * 16

# blob16 column offsets (keys are rotated per-core so xtc = xt[:, 0:CH])
C_I50, C_WV, C_WK, C_WO, C_ONES, C_E4 = 0, 128, 256, 768, 1280, 1408
B16C = 1536
# blob32: rhsm0 0-512, bq 512-516, bk 516-520, bv 520-524, ob row0 524-652
B32C = 652

F32 = mybir.dt.float32
F32R = mybir.dt.float32r
F16 = mybir.dt.float16
AF = mybir.ActivationFunctionType
ALU = mybir.AluOpType

_cache = {}


def _build_program(has_bk=False, has_bv=False):
    nc = bacc.Bacc(None)

    blob16 = nc.dram_tensor("blob16", [128, B16C], F16, kind="ExternalInput")
    blob32 = nc.dram_tensor("blob32", [128, B32C], F32, kind="ExternalInput")
    xt = nc.dram_tensor("xt", [D, N], F16, kind="ExternalInput")
    geo = nc.dram_tensor("geo", [128, N], F16, kind="ExternalInput")
    rhs = nc.dram_tensor("rhs", [128, 6, CH], F16, kind="ExternalInput")
    out = nc.dram_tensor("out", [128, NSUB, D], F32, kind="ExternalOutput")

    with tile.TileContext(nc) as tc, nc.allow_low_precision(
        reason="fp16 operands and f32r rounding are intentional"
    ):
        with (
            tc.tile_pool(name="const", bufs=1) as const,
            tc.tile_pool(name="tmp", bufs=1) as tmp,
            tc.tile_pool(name="s2p", bufs=4) as s2p,
            tc.tile_pool(name="ep", bufs=4) as ep,
            tc.tile_pool(name="gprep", bufs=2) as gprep,
            tc.tile_pool(name="latp", bufs=2) as latp,
        ):
            geo_sb = const.tile([128, N], F16, tag="geo", name="geo_sb")
            rhs_sb = const.tile([128, 6, CH], F16, tag="rhs", name="rhs_sb")
            b16 = const.tile([128, B16C], F16, tag="b16", name="b16")
            xt_sb = const.tile([D, N], F16, tag="xt", name="xt_sb")
            b32 = const.tile([128, B32C], F32, tag="b32", name="b32")
            nc.sync.dma_start(geo_sb[:, 0 : N // 2], geo[:, 0 : N // 2])
            nc.sync.dma_start(rhs_sb[:, 4:6, :], rhs[:, 4:6, :])
            nc.sync.dma_start(rhs_sb[:, 0:4, :], rhs[:, 0:4, :])
            nc.sync.dma_start(geo_sb[:, N // 2 : N], geo[:, N // 2 : N])
            nc.sync.dma_start(b16, blob16[...])
            nc.sync.dma_start(xt_sb[:, 0 : N // 2], xt[:, 0 : N // 2])
            nc.sync.dma_start(xt_sb[:, N // 2 : N], xt[:, N // 2 : N])
            nc.sync.dma_start(b32, blob32[...])

            xtc_sb = xt_sb[:, 0:CH]
            i50_sb = b16[:, C_I50 : C_I50 + D]
            wv_sb = b16[:, C_WV : C_WV + D]
            ones_sb = b16[0:1, C_ONES : C_ONES + D]

            ob_sb = const.tile([1, D], F16, tag="ob", name="ob_sb")
            nc.gpsimd.tensor_copy(ob_sb, b32[0:1, 524 : 524 + D])

            nb2 = const.tile([128, 1], F32, tag="nb2", name="nb2")
            nc.gpsimd.memset(nb2, -2.0)

            qk4 = const.tile([D, H, CH], F16, tag="qk4", name="qk4")
            V = const.tile([128, NJT, H, HD + 1], F16, tag="V", name="V")
            E16 = const.tile([128, CH], F16, tag="E16", name="E16")
            geom16 = const.tile([128, NJT, CH], F16, tag="geom16", name="geom16")
            gate16 = const.tile([128, NJT, CH], F16, tag="gate16", name="gate16")
            headcat = const.tile([HD, H, CH], F16, tag="headcat", name="headcat")

            nc.gpsimd.memset(V[:, :, :, HD : HD + 1], 1.0)

            # ---- main loop: geometry and attention interleaved ----
            with (
                tc.tile_pool(name="a_pv", bufs=1, space="PSUM") as a_pv,
            ):
                pv0 = a_pv.tile([128, CH], F32, tag="pv0", name="pv0")
                pv1 = a_pv.tile([128, CH], F32, tag="pv1", name="pv1")
                pvb = [pv0, pv1]
                prev = [None]

                with (
                    tc.tile_pool(name="g_pa", bufs=1, space="PSUM") as g_pa,
                    tc.tile_pool(name="g_pb", bufs=1, space="PSUM") as g_pb,
                    tc.tile_pool(name="a_sg", bufs=2, space="PSUM") as a_sg,
                ):
                    gpre = [None]

                    # qk4_h = M_h^T @ x_chunk with M_h = SC * Wq_h Wk_h^T
                    # host-folded (kills the q->qT->k chain on device)
                    for h in range(2):
                        sgk = a_sg.tile([128, 2, CH], F32, tag="sg",
                                        name="sgk")
                        for hh in range(2):
                            nc.tensor.matmul(
                                sgk[:, hh, :],
                                lhsT=b16[:, C_WK + (2 * h + hh) * D
                                         : C_WK + (2 * h + hh + 1) * D],
                                rhs=xtc_sb,
                                start=True,
                                stop=True,
                            )
                        nc.scalar.copy(qk4[:, 2 * h : 2 * h + 2, :], sgk)
                    sge = a_sg.tile([128, 2, CH], F32, tag="sg", name="sge")
                    nc.tensor.matmul(
                        sge[:, 0, :], lhsT=ones_sb, rhs=rhs_sb[0:1, 3, :],
                        start=True, stop=True,
                    )
                    nc.scalar.copy(E16, sge[:, 0, :])

                    def emit_g(jt):
                        j0 = jt * 128
                        lhs = geo_sb[:, j0 : j0 + 128]
                        pa1 = g_pa.tile([128, CH], F32, tag="pa", name="pa1")
                        nc.tensor.matmul(pa1, lhsT=lhs, rhs=rhs_sb[:, 4, :],
                                         start=True, stop=False)
                        nc.tensor.matmul(pa1, lhsT=lhs, rhs=rhs_sb[:, 5, :],
                                         start=False, stop=True,
                                         skip_group_check=True)
                        pb = g_pb.tile([128, CH], F32, tag="pb", name="pb")
                        nc.scalar.activation(pb, pa1, AF.Square)
                        pa2 = g_pa.tile([128, CH], F32, tag="pa", name="pa2")
                        nc.tensor.matmul(pa2, lhsT=lhs, rhs=rhs_sb[:, 2, :],
                                         start=True, stop=True)
                        nc.tensor.matmul(
                            pb, lhsT=lhs, rhs=rhs_sb[:, 0, :], start=False,
                            stop=False, skip_group_check=True,
                        )
                        nc.tensor.matmul(
                            pb, lhsT=lhs, rhs=rhs_sb[:, 1, :], start=False,
                            stop=True, skip_group_check=True,
                        )
                        if GEOM_ACT[jt]:
                            nc.scalar.copy(geom16[:, jt, :], pb)
                        else:
                            nc.vector.tensor_copy(geom16[:, jt, :], pb)
                        lata = latp.tile([128, CH], F16, tag="lata",
                                         name="lata")
                        nc.scalar.activation(lata, pa2, AF.Abs)
                        if jt % 2 == 0:
                            gpre[0] = gprep.tile([128, 2, CH], F16, tag="gp",
                                                 name="gpre")
                        nc.vector.tensor_tensor(
                            gpre[0][:, jt % 2, :], lata, E16, ALU.subtract
                        )
                        if jt % 2 == 1:
                            g = jt // 2
                            tgrp = latp.tile([128, 2, CH], F16, tag="tgrp",
                                             name="tgrp")
                            nc.scalar.activation(
                                tgrp, gpre[0], AF.Tanh,
                                scale=-0.5 * GATE_INV_SCALE,
                            )
                            # gate = 0.5 * (tanh + 1)
                            nc.vector.tensor_scalar(
                                gate16[:, g * 2 : g * 2 + 2, :], tgrp, 1.0,
                                0.5, ALU.add, ALU.mult,
                            )

                    def emit_a(jt):
                        s2 = s2p.tile([128, H, CH], F16, tag="s2", name="s2")
                        e = ep.tile([128, H, CH], F16, tag="e", name="e")
                        for half in range(2):
                            sg = a_sg.tile([128, 2, CH], F32, tag="sg",
                                           name="sg")
                            for hh in range(2):
                                h = 2 * half + hh
                                nc.tensor.matmul(
                                    sg[:, hh, :],
                                    lhsT=xt_sb[:, jt * 128 : (jt + 1) * 128],
                                    rhs=qk4[:, h, :],
                                    start=True,
                                    stop=False,
                                )
                                nc.tensor.matmul(
                                    sg[:, hh, :],
                                    lhsT=i50_sb,
                                    rhs=geom16[:, jt, :],
                                    start=False,
                                    stop=True,
                                    skip_group_check=True,
                                )
                            nc.vector.scalar_tensor_tensor(
                                s2[:, 2 * half : 2 * half + 2, :],
                                sg,
                                0.0,
                                gate16[:, jt : jt + 1, :].to_broadcast(
                                    [128, 2, CH]
                                ),
                                ALU.bypass,
                                ALU.mult,
                            )
                            nc.scalar.activation(
                                e[:, 2 * half : 2 * half + 2, :],
                                s2[:, 2 * half : 2 * half + 2, :],
                                AF.Exp, bias=nb2,
                            )
                        if prev[0] is not None:
                            emit_pv(*prev[0])
                        prev[0] = (jt, e)

                    def emit_pv(jt, e):
                        for h in range(H):
                            b0 = (h % 2) * 64
                            nc.tensor.matmul(
                                pvb[h // 2][b0 : b0 + HD + 1, :],
                                lhsT=V[:, jt, h, :],
                                rhs=e[:, h, :],
                                start=(jt == 0),
                                stop=(jt == NJT - 1),
                                skip_group_check=True,
                            )

                    def emit_v(vj):
                        sg = a_sg.tile([128, 2, CH], F32, tag="sg", name="sgv")
                        nc.tensor.matmul(
                            sg[:, 0, 0:D],
                            lhsT=xt_sb[:, vj * 128 : (vj + 1) * 128],
                            rhs=wv_sb,
                            start=True,
                            stop=True,
                        )
                        nc.vector.tensor_copy(
                            out=V[:, vj, :, 0:HD],
                            in_=sg[:, 0, 0:D].rearrange("p (h d) -> p h d",
                                                        h=H),
                        )

                    vper = (NJT + LAG - 1) // LAG  # V-projections per G step
                    for jt in range(NJT):
                        emit_g(jt)
                        if jt < LAG:
                            for vj in range(jt * vper,
                                            min((jt + 1) * vper, NJT)):
                                emit_v(vj)
                        else:
                            emit_a(jt - LAG)
                    for jt in range(NJT - LAG, NJT):
                        emit_a(jt)
                    emit_pv(*prev[0])

                # ---- finish: normalize, concat heads, project ----
                with tc.tile_pool(name="f_ps", bufs=2, space="PSUM") as f_ps:
                    recips = []
                    for h in range(H):
                        b0 = (h % 2) * 64
                        recip = tmp.tile([1, CH], F16, tag="recip",
                                         name="recip", bufs=4)
                        nc.vector.reciprocal(
                            recip, pvb[h // 2][b0 + HD : b0 + HD + 1, :]
                        )
                        recips.append(recip)
                    bc16s = []
                    for pair in range(2):
                        bc_ps = f_ps.tile([2 * HD, CH], F32, tag="bc",
                                          name="bc_ps")
                        for hh in range(2):
                            nc.tensor.matmul(
                                bc_ps[hh * HD : (hh + 1) * HD, :],
                                lhsT=ones_sb[0:1, 0:HD],
                                rhs=recips[2 * pair + hh],
                                start=True,
                                stop=True,
                            )
                        bc16 = tmp.tile([2 * HD, CH], F16, tag="bc16",
                                        name="bc16", bufs=2)
                        nc.scalar.copy(bc16, bc_ps)
                        bc16s.append(bc16)
                    for h in range(H):
                        b0 = (h % 2) * 64
                        nc.vector.scalar_tensor_tensor(
                            headcat[:, h, :],
                            pvb[h // 2][b0 : b0 + HD, :],
                            0.0,
                            bc16s[h // 2][(h % 2) * HD : (h % 2 + 1) * HD, :],
                            ALU.bypass,
                            ALU.mult,
                        )
                        if has_bv:
                            nc.scalar.activation(
                                headcat[:, h, :], headcat[:, h, :],
                                AF.Identity,
                                bias=b32[0:HD, 520 + h : 521 + h],
                            )

                    f_all = tmp.tile([128, NSUB, D], F32, tag="fall",
                                     name="f_all")
                    for s in range(NSUB):
                        fps = f_ps.tile([128, D], F32, tag="f", name="fps")
                        for h in range(H):
                            nc.tensor.matmul(
                                fps,
                                lhsT=headcat[:, h, s * 128 : (s + 1) * 128],
                                rhs=b16[0:HD, C_WO + h * D : C_WO + (h + 1) * D],
                                start=(h == 0),
                                stop=False,
                            )
                        nc.tensor.matmul(
                            fps, lhsT=ones_sb, rhs=ob_sb, start=False,
                            stop=True,
                        )
                        if s % 2:
                            nc.scalar.copy(f_all[:, s, :], fps)
                        else:
                            nc.vector.tensor_copy(f_all[:, s, :], fps)
                    nc.sync.dma_start(out[...], f_all)

    nc.finalize()
    return nc


def _split_hi_lo(v):
    """Split fp32 array into an fp16-exact hi part and the fp32 residual."""
    v = np.asarray(v, np.float32)
    hi = v.astype(np.float16).astype(np.float32)
    lo = (v.astype(np.float64) - hi).astype(np.float32)
    return hi, lo


_exp_basis = None


def _get_exp_basis():
    """Separable rank-R_EXP expansion of exp(-|a-b|) on [0,1]^2."""
    global _exp_basis
    if _exp_basis is None:
        g = np.linspace(0.0, 1.0, 2048)
        K = np.exp(-np.abs(g[:, None] - g[None, :]))
        U, s, Vt = np.linalg.svd(K)
        r = R_EXP
        FI = U[:, :r] * np.sqrt(s[:r])
        GJ = Vt[:r].T * np.sqrt(s[:r])
        _exp_basis = (g, FI, GJ)
    return _exp_basis


def _prep_core_inputs(inputs, core):
    b, ch = core // 4, core % 4
    i0 = ch * CH
    x = np.ascontiguousarray(inputs["x"][b], np.float32)  # [N, D]
    pdir = np.ascontiguousarray(inputs["principal_dir"][b], np.float32)
    nrm = np.ascontiguousarray(inputs["normals"][b], np.float32)
    crv = inputs["curvature"][b].astype(np.float32)
    dens = inputs["density"][b].astype(np.float32)
    lin = inputs["linearity"][b].astype(np.float32)
    qkv_w = inputs["qkv_w"].astype(np.float32)
    qkv_b = inputs["qkv_b"].astype(np.float32)
    out_w = inputs["out_w"].astype(np.float32)

    xyz = x[:, :3]
    n2 = (xyz.astype(np.float64) ** 2).sum(-1).astype(np.float32)
    cr = np.cross(pdir, nrm)
    side = cr / (np.linalg.norm(cr, axis=-1, keepdims=True) + 1e-8)
    rowdot = (xyz * pdir).sum(-1)
    rowsidedot = (xyz * side).sum(-1)

    xhi, xlo = _split_hi_lo(xyz)
    n2hi, n2lo = _split_hi_lo(n2)
    shi, slo = _split_hi_lo(side)
    rdhi, rdlo = _split_hi_lo(rowdot)
    rshi, rslo = _split_hi_lo(rowsidedot)

    ci = crv[i0 : i0 + CH]
    di = dens[i0 : i0 + CH]
    li = lin[i0 : i0 + CH]
    s_i = np.sqrt(1.0 - 0.25 * (1.0 - li)).astype(np.float32)

    grid, FI, GJ = _get_exp_basis()
    gj = np.stack([np.interp(crv, grid, GJ[:, k]) for k in range(R_EXP)])
    fi = np.stack([np.interp(ci, grid, FI[:, k]) for k in range(R_EXP)])
    gfac = (GAMMA / FOLD) * dens  # j-side factor

    # keys are rotated so this core's queries sit at columns 0:CH
    perm = (np.arange(N) + i0) % N

    # GEO rows (fp16): 0-2 xhi_j, 3-5 xlo_j, 6 n2hi, 7 n2lo, 8 ones,
    # 13-15 xhi dup, 16 ones dup, 17.. expansion g-side
    geo = np.zeros((128, N), np.float32)
    geo[0:3] = xhi.T
    geo[3:6] = xlo.T
    geo[6] = n2hi
    geo[7] = n2lo
    geo[8] = 1.0
    geo[13:16] = xhi.T
    geo[16] = 1.0
    for k in range(R_EXP):
        geo[17 + 3 * k : 20 + 3 * k] = (gj[k] * gfac)[None, :] * nrm.T
    geo = geo[:, perm]

    # rhs m0 (f32): dp' = s_i * (rowdot_i - x_j . pdir_i)
    phi, plo = _split_hi_lo(pdir[i0 : i0 + CH])
    rhsm0 = np.zeros((128, CH), np.float32)
    rhsm0[0:3] = -phi.T * s_i
    rhsm0[3:6] = -phi.T * s_i
    rhsm0[13:16] = -plo.T * s_i
    rhsm0[8] = rdhi[i0 : i0 + CH] * s_i
    rhsm0[16] = rdlo[i0 : i0 + CH] * s_i

    rhsf = np.zeros((128, 6, CH), np.float32)
    # m-slot 0: -d2 (exact negation)
    xhic = xhi[i0 : i0 + CH]
    xloc = xlo[i0 : i0 + CH]
    rhsf[0:3, 0] = 2.0 * xhic.T
    rhsf[3:6, 0] = 2.0 * xhic.T
    rhsf[13:16, 0] = 2.0 * xloc.T
    rhsf[6, 0] = -1.0
    rhsf[7, 0] = -1.0
    rhsf[8, 0] = -n2hi[i0 : i0 + CH]
    rhsf[16, 0] = -n2lo[i0 : i0 + CH]
    # m-slot 1: expansion f-side
    nic = nrm[i0 : i0 + CH]
    for k in range(R_EXP):
        rhsf[17 + 3 * k : 20 + 3 * k, 1] = fi[k][None, :] * nic.T
    # m-slot 3 row 0: E_i for the gate
    rhsf[0, 3] = HALF_W * (0.5 + di)
    # m-slots 4/5: dp' rhs as fp16 hi + lo
    m0hi = rhsm0.astype(np.float16).astype(np.float32)
    rhsf[:, 4] = m0hi
    rhsf[:, 5] = rhsm0 - m0hi
    # m-slot 2: lateral = rowsidedot_i - x_j . side_i
    sh, sl = shi[i0 : i0 + CH], slo[i0 : i0 + CH]
    rhsf[0:3, 2] = -sh.T
    rhsf[3:6, 2] = -sh.T
    rhsf[13:16, 2] = -sl.T
    rhsf[8, 2] = rshi[i0 : i0 + CH]
    rhsf[16, 2] = rslo[i0 : i0 + CH]
    xT = np.ascontiguousarray(x.T)[:, perm]
    f16 = np.float16

    blob16 = np.zeros((128, B16C), f16)
    blob16[:, C_I50 : C_I50 + D] = (FOLD * np.eye(D, dtype=np.float32)).astype(
        f16
    )
    blob16[:, C_WV : C_WV + D] = qkv_w[:, 2 * D : 3 * D].astype(f16)
    wqh = qkv_w[:, 0:D].reshape(D, H, HD).astype(np.float64)
    wkh = qkv_w[:, D : 2 * D].reshape(D, H, HD).astype(np.float64)
    for h in range(H):
        m_h = SC * (wqh[:, h, :] @ wkh[:, h, :].T)  # [D, D]
        blob16[:, C_WK + h * D : C_WK + (h + 1) * D] = m_h.astype(f16)
    wo_a = out_w.reshape(H, HD, D).transpose(1, 0, 2)
    blob16[0:HD, C_WO : C_WO + H * D] = wo_a.reshape(HD, H * D).astype(f16)
    blob16[0:1, C_ONES : C_ONES + D] = 1.0
    # E4 block-ones for the finish bc broadcast
    for r in range(H):
        blob16[r, C_E4 + r * HD : C_E4 + (r + 1) * HD] = 1.0

    blob32 = np.zeros((128, B32C), np.float32)
    blob32[0:HD, 512:516] = (qkv_b[0:D] * SC).reshape(H, HD).T
    blob32[0:HD, 516:520] = qkv_b[D : 2 * D].reshape(H, HD).T
    blob32[0:HD, 520:524] = qkv_b[2 * D : 3 * D].reshape(H, HD).T
    blob32[0:1, 524 : 524 + D] = inputs["out_b"].astype(np.float32)[None, :]

    return {
        "blob16": blob16,
        "blob32": blob32,
        "xt": xT.astype(f16),
        "geo": geo.astype(f16),
        "rhs": rhsf.astype(f16),
    }


def _run(inputs, trace=False):
    has_bk = bool(np.any(inputs["qkv_b"][D : 2 * D]))
    has_bv = bool(np.any(inputs["qkv_b"][2 * D : 3 * D]))
    key = ("nc", has_bk, has_bv)
    if key not in _cache:
        _cache[key] = _build_program(has_bk, has_bv)
    nc = _cache[key]
    in_maps = [_prep_core_inputs(inputs, c) for c in range(NCORES)]
    res = run_bass_kernel_spmd(nc, in_maps, core_ids=list(range(NCORES)), trace=trace)
    full = np.empty((B, N, D), np.float32)
    for c in range(NCORES):
        b, ch = c // 4, c % 4
        o = res.results[c]["out"]  # [128, NSUB, D]
        full[b, ch * CH : (ch + 1) * CH, :] = o.transpose(1, 0, 2).reshape(
            CH, D
        )
    return full, res


def kernel(**inputs):
    out, _ = _run(inputs)
    return out
